# revision 56
# baseline (speedup 1.0000x reference)
"""MoE layer (E=8 experts, top-2 routing) on 8 Trainium2 NeuronCores.

Strategy: expert-parallel with a 2-slot load-balancing template. The host
computes the gating network in fp64 (logits = x @ wg + bg, top-2, softmax)
and dispatches token-slots to cores. Each core's SPMD program processes
  slot0: NT0 tokens with weight set A (the core's primary expert), then
  slot1: NT1=128 tokens with weight set B (a top-up block of whichever
         expert overflowed NT0 tokens -- host-assigned).
This pads every core to NT0+128 tokens instead of the global max expert
count rounded up (4224 vs 4480 for the reference input), cutting PE time.

Per core FFN:  y = relu(x_e @ w1[e] + b1[e]) @ w2[e], then rows scaled by
the gate weight on-device; the host scatter-adds the two slots per token
back together (plus the combine@b2 bias term).

Mixed precision: per core, slot0 is sorted by gate weight descending and
the last NF=1920 slots (the lowest-gate ones, all g<=0.5, as
384+512+512+256+256-token chunks) run mm1 with the d<256 contraction
half as fp8-e4m3 DoubleRow matmuls (2x PE throughput; quantization noise
there is damped by the small gate); the final 256-token chunk (lowest
gates of all) additionally runs mm2's f<512 half in fp8, with h written
directly in fp8 by the mm1 activation and the 2^15 psum scale folded
into host-pre-scaled gates. Measured rel-err 1.83e-2 against the 2e-2
gate (bf16-only is 3.9e-3; 384 mm2-fp8 tokens would reach ~1.98e-2,
NF=2048 mm1 would cross ~2.2e-2). The bf16 halves carry a 2^15 weight
pre-scale so one activation scale (2^-15) serves both; bf16 matmuls must
run FIRST in each psum accumulation group -- the reverse order returns
wrong psum contents on hardware.

Other hardware notes baked into the schedule:
  - PE DVFS: full clock arrives ~6us after first PE activity and decays
    on ~1us gaps, so dummy warmup matmuls (on a vector-memset tile) run
    from the engine-init floor (~8us) until the chunk0 DMA set lands
    (~13.3us). gpsimd engagement depresses the PE clock ~20% kernel-wide
    -- do not use it.
  - mm1 runs one chunk ahead of mm2 so the first mm2's w2a dependency
    has ~2 chunk-times of DMA slack.
  - The last 128-token block's mm2 is column-split into four quarters,
    alternating store rings, to shrink the post-last-matmul tail.

DMA plan (two HWDGE FIFO rings; each dma_start costs ~600ns of serial
descriptor-gen on its sequencer and SDMA execution begins ~8.2us in):
  scalar ring: x chunk0 lo-half, w1a fb0, b1a, gates, w2a(A)
  sync ring:   x0 hi-half, w1a fb1-7, x1, w2a(B), x2.., fp8 tiles,
               w1b, w2b, b1b, then output stores

All device inputs are host-permuted so every SBUF partition's data is one
contiguous DRAM run. Hardcoded problem shape: x [4,4096,512],
w1 [8,512,1024], w2 [8,1024,512], wg [512,8], top_k=2.
"""

import os
import numpy as np

B, S, D, F, E = 4, 4096, 512, 1024, 8
TOP_K = 2
N_CORES = 8
KD = D // 128   # contraction blocks for mm1
FB = F // 128   # F blocks (h partition blocks / mm2 contraction blocks)
NT1 = 128       # top-up slot tokens

TRACE = os.environ.get("MOE_TRACE", "0") == "1"

_PROGRAM_CACHE = {}


def _chunk_plan(NT0, nf=0):
    """Token chunk sizes: slot0 split into bf16 chunks (NT0-nf tokens:
    512s + one 128-multiple remainder) followed by fp8 chunks (nf tokens:
    one 128-multiple remainder + 512s), then the 128-token top-up chunk
    last (small tail). Returns (chunks, fp8_ids)."""
    bf = NT0 - nf
    chunks = [512] * (bf // 512)
    if bf % 512:
        chunks.append(bf % 512)
    nbf = len(chunks)
    if nf % 512:
        chunks.append(nf % 512)
    chunks += [512] * (nf // 512)
    # carve the last 256 fp8 tokens (the lowest gates of all) into their
    # own chunk whose mm2 also runs its f<512 contraction half in fp8
    m2_ids = []
    if nf >= 512 and chunks[-1] == 512:
        chunks[-1] = 256
        chunks.append(256)
        m2_ids = [len(chunks) - 1]
    fp8_ids = list(range(nbf, len(chunks)))
    chunks.append(NT1)
    return chunks, fp8_ids, m2_ids


def _build_program(NT0, nf):
    from concourse import bacc, tile, mybir

    dt = mybir.dt
    DT = dt.bfloat16

    nc = bacc.Bacc("TRN2", target_bir_lowering=False, debug=False)

    chunks, fp8_ids, m2_ids = _chunk_plan(NT0, nf)
    offs = [sum(chunks[:i]) for i in range(len(chunks) + 1)]
    NT = NT0 + NT1
    NTG = NT // 128
    n0 = len(chunks) - 1  # number of slot0 chunks
    m_fp8 = len(fp8_ids)

    # host-permuted inputs: per-partition contiguous runs
    # xp: per chunk c, [128, KD*cs] block at col KD*offs[c]
    xp_d = nc.dram_tensor("xp", [128, KD * NT], DT, kind="ExternalInput").ap()
    # w1a/w1b: fb-major: col = fb*(KD*128) + kc*128 + j
    w1a_d = nc.dram_tensor("w1a", [128, FB * KD * 128], DT, kind="ExternalInput").ap()
    w1b_d = nc.dram_tensor("w1b", [128, FB * KD * 128], DT, kind="ExternalInput").ap()
    # w2a/w2b: col = fb*D + d, partition p = f within fb block
    w2a_d = nc.dram_tensor("w2a", [128, FB * D], DT, kind="ExternalInput").ap()
    w2b_d = nc.dram_tensor("w2b", [128, FB * D], DT, kind="ExternalInput").ap()
    b1a_d = nc.dram_tensor("b1a", [128, FB], dt.float32, kind="ExternalInput").ap()
    b1b_d = nc.dram_tensor("b1b", [128, FB], dt.float32, kind="ExternalInput").ap()
    g_d = nc.dram_tensor("gate2", [128, NTG], dt.float32, kind="ExternalInput").ap()
    y_d = nc.dram_tensor("y", [NT, D], DT, kind="ExternalOutput").ap()
    if m_fp8:
        # fp8 chunk: x8[p, i*cs + t] = x[d=i*128+p, t]*32 (e4m3); chunks
        # packed back-to-back (2*cs cols each)
        xp8_d = nc.dram_tensor("xp8", [128, 2 * nf], dt.float8e4,
                               kind="ExternalInput").ap()
        # w18[p, fb*256 + i*128 + j] = w1[i*128+p, fb*128+j]*1024 (e4m3)
        w18_d = nc.dram_tensor("w18", [128, FB * 256], dt.float8e4,
                               kind="ExternalInput").ap()
        # w1ahi[p, fb*256 + kc*128 + j] = w1[(kc+2)*128+p, fb*128+j]*2^15
        w1ahi_d = nc.dram_tensor("w1ahi", [128, FB * 256], DT,
                                 kind="ExternalInput").ap()
    if m2_ids:
        # mm2-fp8 weights: w28[p, j*1024 + i*512 + d] = w2[(2j+i)*128+p, d]
        # * 1024 (e4m3, f<512); w2hi[p, k*512 + d] = w2[512+k*128+p, d]*2^15
        w28_d = nc.dram_tensor("w28", [128, 2048], dt.float8e4,
                               kind="ExternalInput").ap()
        w2hi_d = nc.dram_tensor("w2hi", [128, 4 * 512], DT,
                                kind="ExternalInput").ap()
        # b1 pre-scaled by SH=32 for the fp8-h activation
        b1a8_d = nc.dram_tensor("b1a8", [128, FB], dt.float32,
                                kind="ExternalInput").ap()

    with tile.TileContext(nc) as tc:
        with (
            tc.tile_pool(name="sb", bufs=1) as sbpool,
            tc.tile_pool(name="ps", bufs=4, space="PSUM") as pspool,
        ):
            wpool = xpool = sbpool
            ps1 = ps2 = pspool
            # ---- head DMA plan. SDMA execution only begins at ~8.2us and
            # early per-ring bandwidth is only ~64GB/s, so the first-chunk
            # critical mass (x0 + w1a fb blocks, ~1.5MB) is balanced across
            # both rings with completion granularity matching the chain
            # consumption order: scalar carries x0lo + w1afb0, sync carries
            # x0hi then per-fb w1a singles.
            cs0 = chunks[0]
            x0 = xpool.tile([128, KD * cs0], DT, tag="x0", name="x0")
            nc.scalar.dma_start(out=x0[0:64, :], in_=xp_d[0:64, 0:KD * cs0])
            nc.sync.dma_start(out=x0[64:128, :], in_=xp_d[64:128, 0:KD * cs0])
            w1a_fb = []
            t = wpool.tile([128, KD * 128], DT, tag="w1a_fb0", name="w1afb0")
            nc.scalar.dma_start(out=t[:], in_=w1a_d[:, 0:KD * 128])
            w1a_fb.append(t)
            for fb in range(1, FB):
                t = wpool.tile([128, KD * 128], DT, tag=f"w1a_fb{fb}",
                               name=f"w1afb{fb}")
                nc.sync.dma_start(
                    out=t[:], in_=w1a_d[:, fb * KD * 128:(fb + 1) * KD * 128])
                w1a_fb.append(t)

            def w1a_slice(fb, kc):
                return w1a_fb[fb][:, kc * 128:(kc + 1) * 128]

            # scalar ring continues: tiny b1a/g (needed by the first RELU),
            # then w2a for the first mm2
            b1a_sb = wpool.tile([128, FB], dt.float32)
            nc.scalar.dma_start(out=b1a_sb[:], in_=b1a_d[:])
            g_sb = wpool.tile([128, NTG], dt.float32)
            nc.scalar.dma_start(out=g_sb[:], in_=g_d[:])
            w2a_A = wpool.tile([128, (FB // 2) * D], DT, name="w2aA")
            nc.scalar.dma_start(out=w2a_A[:], in_=w2a_d[:, 0:(FB // 2) * D])

            # warmup: dummy matmuls on a vector-memset scratch tile keep the
            # PE busy from the engine-init floor (~8.2us) through the DVFS
            # ramp (full clock arrives ~6us after PE-busy-start) until the
            # first x/w tiles land (~11.4us); the scratch psum is never
            # read. vector memset: gpsimd engagement was measured to depress
            # the PE clock ~20% for the whole kernel, and scalar/sync must
            # not be delayed since they issue the DMA descriptor gens.
            # 2 big + 72 small dummies bridge the PE from the engine-init
            # floor (~8.0us) to the ~14.2us arrival of the chunk0 critical
            # mass: bigs at low clock ~1.1us, smalls at 107ns until the
            # clock maxes (~11us), 56ns after. Ending early costs a clock
            # drop (~2us re-ramp); ending late costs one small dummy.
            warm = wpool.tile([128, 512], DT)
            nc.vector.memset(warm[:], 0.0)
            for i in range(54):
                pw = ps2.tile([128, 512], dt.float32, tag="ps2", bufs=5)
                if i < 2:
                    nc.tensor.matmul(pw[:], warm[:, 0:128], warm[:],
                                     start=True, stop=True)
                else:
                    nc.tensor.matmul(pw[:, 0:128], warm[:, 0:128],
                                     warm[:, 0:128], start=True, stop=True)

            # ---- sync (SP) HWDGE ring: bulk loads continue, stores below.
            x_tiles = [x0]

            for c in range(1, len(chunks)):
                cs = chunks[c]
                if c == 1:
                    # two kc-half tiles in the same ring/FIFO slot so
                    # chunk1's mm1 can start on the first half (kc blocks
                    # are read in order)
                    xa = xpool.tile([128, 2 * cs], DT, tag="x1a", name="x1a")
                    nc.sync.dma_start(
                        out=xa[:], in_=xp_d[:, KD * offs[c]:KD * offs[c] + 2 * cs])
                    xb = xpool.tile([128, 2 * cs], DT, tag="x1b", name="x1b")
                    nc.sync.dma_start(
                        out=xb[:],
                        in_=xp_d[:, KD * offs[c] + 2 * cs:KD * (offs[c] + cs)])
                    x_tiles.append((xa, xb))
                elif c == 2:
                    # w2a_B rides between x1 and x2: needed by mm2(c0) which
                    # now runs after mm1(c1), so ~21us of slack
                    w2a_B = wpool.tile([128, (FB // 2) * D], DT, name="w2aB")
                    nc.sync.dma_start(out=w2a_B[:], in_=w2a_d[:, (FB // 2) * D:])
                    xt = xpool.tile([128, KD * cs], DT, tag=f"x{c}", name=f"x{c}")
                    nc.sync.dma_start(
                        out=xt[:], in_=xp_d[:, KD * offs[c]:KD * (offs[c] + cs)])
                    x_tiles.append(xt)
                elif c in fp8_ids:
                    # fp8 chunk: only the kc2-3 (d>=256) half comes from xp;
                    # the d<256 half arrives as fp8 via xp8 below
                    xt = xpool.tile([128, 2 * cs], DT, tag=f"x{c}", name=f"x{c}")
                    nc.sync.dma_start(
                        out=xt[:],
                        in_=xp_d[:, KD * offs[c] + 2 * cs:KD * (offs[c] + cs)])
                    x_tiles.append(xt)
                else:
                    xt = xpool.tile([128, KD * cs], DT, tag=f"x{c}", name=f"x{c}")
                    nc.sync.dma_start(
                        out=xt[:], in_=xp_d[:, KD * offs[c]:KD * (offs[c] + cs)])
                    x_tiles.append(xt)

            x8_tiles = {}
            if m_fp8:
                x8off = 0
                for c in fp8_ids:
                    cs8 = chunks[c]
                    t8 = xpool.tile([128, 2, cs8], dt.float8e4, name=f"x8c{c}")
                    nc.sync.dma_start(
                        out=t8[:], in_=xp8_d[:, x8off:x8off + 2 * cs8]
                        .rearrange("p (a b) -> p a b", a=2))
                    x8_tiles[c] = t8
                    x8off += 2 * cs8
                w18_sb = wpool.tile([128, FB, 2, 128], dt.float8e4, name="w18")
                nc.sync.dma_start(
                    out=w18_sb[:],
                    in_=w18_d[:].rearrange("p (f a b) -> p f a b", f=FB, a=2))
                w1ahi_sb = wpool.tile([128, FB, 2, 128], DT, name="w1ahi")
                nc.sync.dma_start(
                    out=w1ahi_sb[:],
                    in_=w1ahi_d[:].rearrange("p (f a b) -> p f a b", f=FB, a=2))
            if m2_ids:
                w28_sb = wpool.tile([128, 2, 2, 512], dt.float8e4, name="w28")
                nc.sync.dma_start(
                    out=w28_sb[:],
                    in_=w28_d[:].rearrange("p (j i d) -> p j i d", j=2, i=2))
                w2hi_sb = wpool.tile([128, 4, 512], DT, name="w2hi")
                nc.sync.dma_start(
                    out=w2hi_sb[:],
                    in_=w2hi_d[:].rearrange("p (k d) -> p k d", k=4))
                b1a8_sb = wpool.tile([128, FB], dt.float32)
                nc.sync.dma_start(out=b1a8_sb[:], in_=b1a8_d[:])

            if len(chunks) <= 2:  # tiny-NT0 fallback: w2a_B not yet emitted
                w2a_B = wpool.tile([128, (FB // 2) * D], DT, name="w2aB")
                nc.sync.dma_start(out=w2a_B[:], in_=w2a_d[:, (FB // 2) * D:])

            w1b_sb = wpool.tile([128, FB * KD * 128], DT)
            nc.sync.dma_start(out=w1b_sb[:], in_=w1b_d[:])
            w2b_sb = wpool.tile([128, FB * D], DT)
            nc.sync.dma_start(out=w2b_sb[:], in_=w2b_d[:])
            b1b_sb = wpool.tile([128, FB], dt.float32)
            nc.sync.dma_start(out=b1b_sb[:], in_=b1b_d[:])

            def w1_slice(c, fb, kc):
                if c < n0:
                    return w1a_slice(fb, kc)
                return w1b_sb[:, fb * KD * 128 + kc * 128:fb * KD * 128 + (kc + 1) * 128]

            def w2_slice(c, fb, c0=0, c1=D):
                if c >= n0:
                    return w2b_sb[:, fb * D + c0:fb * D + c1]
                t, f = (w2a_A, fb) if fb < FB // 2 else (w2a_B, fb - FB // 2)
                return t[:, f * D + c0:f * D + c1]

            h_tiles = {}

            def do_mm1(c):
                cs = chunks[c]
                x_sb = x_tiles[c]
                b1_sb = b1a_sb if c < n0 else b1b_sb
                h_sb = sbpool.tile([128, FB, cs], DT, tag="h", bufs=4)
                h_tiles[c] = h_sb
                for fb in range(FB):
                    p = ps1.tile([128, cs], dt.float32, tag="ps1", bufs=3)
                    for kc in range(KD):
                        if isinstance(x_sb, tuple) and len(x_sb) == KD:
                            xop = x_sb[kc][:]
                        elif isinstance(x_sb, tuple):
                            xt_, k_ = (x_sb[0], kc) if kc < 2 else (x_sb[1], kc - 2)
                            xop = xt_[:, k_ * cs:(k_ + 1) * cs]
                        else:
                            xop = x_sb[:, kc * cs:(kc + 1) * cs]
                        nc.tensor.matmul(
                            p[:],
                            w1_slice(c, fb, kc),
                            xop,
                            start=(kc == 0),
                            stop=(kc == KD - 1),
                        )
                    nc.scalar.activation(
                        h_sb[:, fb, :],
                        p[:],
                        mybir.ActivationFunctionType.Relu,
                        bias=b1_sb[:, fb:fb + 1],
                        scale=1.0,
                    )

            def do_mm1_fp8(c):
                # d<256 half of the contraction as one DoubleRow fp8 matmul
                # per (fb, token-half); d>=256 half in bf16 with weights
                # pre-scaled by 2^15 to match the fp8 product scale
                # (32*1024); the activation divides the sum back out.
                cs = chunks[c]
                x_hi = x_tiles[c]       # [128, 2*cs] bf16: kc2,kc3
                x_lo = x8_tiles[c]      # [128, 2, cs] fp8
                m2 = c in m2_ids
                if m2:
                    # h split by dtype: fb0-3 as fp8*32 (feeds mm2's DR
                    # half), fb4-7 bf16
                    h8_sb = sbpool.tile([128, 4, cs], dt.float8e4, tag="h8",
                                        bufs=2)
                    hb_sb = sbpool.tile([128, 4, cs], DT, tag="hb", bufs=2)
                    h_tiles[c] = (h8_sb, hb_sb)
                else:
                    h_sb = sbpool.tile([128, FB, cs], DT, tag="h", bufs=4)
                    h_tiles[c] = h_sb
                ths = [(t0, min(256, cs - t0)) for t0 in range(0, cs, 256)]
                for fb in range(FB):
                    p = ps1.tile([128, cs], dt.float32, tag="ps1", bufs=3)
                    # bf16 half FIRST, DR fp8 accumulating after: the
                    # reverse order (DR with start, bf16 accumulating)
                    # produces wrong psum contents on hardware
                    for kc in range(2):
                        nc.tensor.matmul(
                            p[:],
                            w1ahi_sb[:, fb, kc],
                            x_hi[:, kc * cs:(kc + 1) * cs],
                            start=(kc == 0), stop=False,
                            skip_group_check=True,
                        )
                    for ti, (t0, tw) in enumerate(ths):
                        nc.tensor.matmul(
                            p[:, t0:t0 + tw],
                            w18_sb[:, fb],
                            x_lo[:, :, t0:t0 + tw],
                            start=False, stop=(ti == len(ths) - 1),
                            perf_mode=mybir.MatmulPerfMode.DoubleRow,
                            skip_group_check=True,
                        )
                    if m2 and fb < 4:
                        # h8 = relu(pre*32): fold SH into the act scale
                        # (relu is positively homogeneous); bias = b1*32
                        nc.scalar.activation(
                            h8_sb[:, fb, :],
                            p[:],
                            mybir.ActivationFunctionType.Relu,
                            bias=b1a8_sb[:, fb:fb + 1],
                            scale=32.0 / 32768.0,
                        )
                    elif m2:
                        nc.scalar.activation(
                            hb_sb[:, fb - 4, :],
                            p[:],
                            mybir.ActivationFunctionType.Relu,
                            bias=b1a_sb[:, fb:fb + 1],
                            scale=1.0 / 32768.0,
                        )
                    else:
                        nc.scalar.activation(
                            h_sb[:, fb, :],
                            p[:],
                            mybir.ActivationFunctionType.Relu,
                            bias=b1a_sb[:, fb:fb + 1],
                            scale=1.0 / 32768.0,
                        )

            def do_mm2_fp8(c):
                # f<512 contraction half as DoubleRow fp8 (h8*32 x w2*1024),
                # f>=512 in bf16 with w2 pre-scaled 2^15; psum = 2^15 * y,
                # compensated by host-pre-scaled gates for these blocks.
                cs = chunks[c]
                h8_sb, hb_sb = h_tiles.pop(c)
                for tb in range(cs // 128):
                    blk = offs[c] // 128 + tb
                    r0 = offs[c] + tb * 128
                    p2 = ps2.tile([128, 512], dt.float32, tag="ps2", bufs=5)
                    for k in range(4):  # bf16 first (fb4-7)
                        nc.tensor.matmul(
                            p2[:],
                            hb_sb[:, k, tb * 128:(tb + 1) * 128],
                            w2hi_sb[:, k],
                            start=(k == 0), stop=False,
                            skip_group_check=True,
                        )
                    for j in range(2):
                        for q in range(2):
                            nc.tensor.matmul(
                                p2[:, q * 256:(q + 1) * 256],
                                h8_sb[:, 2 * j:2 * j + 2, tb * 128:(tb + 1) * 128],
                                w28_sb[:, j, :, q * 256:(q + 1) * 256],
                                start=False, stop=(j == 1 and q == 1),
                                perf_mode=mybir.MatmulPerfMode.DoubleRow,
                                skip_group_check=True,
                            )
                    o_sb = sbpool.tile([128, 512], DT, tag="o", bufs=10)
                    nc.vector.tensor_scalar_mul(
                        o_sb[:], p2[:], g_sb[:, blk:blk + 1]
                    )
                    nc.sync.dma_start(out=y_d[r0:r0 + 128, :], in_=o_sb[:])

            def do_mm2(c):
                cs = chunks[c]
                h_sb = h_tiles.pop(c)
                last_chunk = c == len(chunks) - 1
                for tb in range(cs // 128):
                    blk = offs[c] // 128 + tb
                    r0 = offs[c] + tb * 128
                    if not (last_chunk and tb == cs // 128 - 1):
                        p2 = ps2.tile([128, 512], dt.float32, tag="ps2", bufs=5)
                        for fb in range(FB):
                            nc.tensor.matmul(
                                p2[:],
                                h_sb[:, fb, tb * 128:(tb + 1) * 128],
                                w2_slice(c, fb),
                                start=(fb == 0),
                                stop=(fb == FB - 1),
                            )
                        o_sb = sbpool.tile([128, 512], DT, tag="o", bufs=10)
                        nc.vector.tensor_scalar_mul(
                            o_sb[:], p2[:], g_sb[:, blk:blk + 1]
                        )
                        nc.sync.dma_start(out=y_d[r0:r0 + 128, :], in_=o_sb[:])
                    else:
                        # final 128-token block: column-split mm2 into four
                        # quarters so the gate-scale + store of earlier
                        # quarters overlap mm2 of later ones, shrinking the
                        # post-last-matmul tail (store descriptor-gen is
                        # ~600ns serial per ring, so alternate rings)
                        for q, eng in ((0, nc.sync), (1, nc.scalar),
                                       (2, nc.sync), (3, nc.scalar)):
                            # reuse the regular ps2 slots ([128,512] tag) so
                            # PSUM stays within the 8-bank budget
                            p2 = ps2.tile([128, 512], dt.float32, tag="ps2", bufs=5)
                            for fb in range(FB):
                                nc.tensor.matmul(
                                    p2[:, 0:128],
                                    h_sb[:, fb, tb * 128:(tb + 1) * 128],
                                    w2_slice(c, fb, q * 128, (q + 1) * 128),
                                    start=(fb == 0),
                                    stop=(fb == FB - 1),
                                )
                            o_sb = sbpool.tile([128, 128], DT, tag="oh", bufs=4)
                            nc.vector.tensor_scalar_mul(
                                o_sb[:], p2[:, 0:128], g_sb[:, blk:blk + 1]
                            )
                            eng.dma_start(
                                out=y_d[r0:r0 + 128, q * 128:(q + 1) * 128],
                                in_=o_sb[:],
                            )

            # software pipeline: mm1 runs one chunk ahead of mm2, so the
            # first mm2's w2a dependency has ~2 chunk-times of DMA slack
            def do_mm2_any(c):
                if c in m2_ids:
                    do_mm2_fp8(c)
                else:
                    do_mm2(c)

            def touch(c):
                # absorb the x-tile DMA-semaphore wait (and its PE pipeline
                # break, ~200ns) on a tiny dummy matmul one chunk early,
                # so the chunk's first real matmul issues back-to-back
                tiles = []
                if c in x8_tiles:
                    tiles.append(x8_tiles[c][:, 0:1, 0:1])
                xs = x_tiles[c]
                if isinstance(xs, tuple):
                    tiles += [t[:, 0:1] for t in xs]
                else:
                    tiles.append(xs[:, 0:1])
                for ap in tiles:
                    pt = ps1.tile([128, 512], dt.float32, tag="ps1", bufs=3)
                    nc.tensor.matmul(pt[0:1, 0:1], ap, ap,
                                     start=True, stop=True,
                                     skip_group_check=True)

            nchunks = len(chunks)
            for c in range(nchunks):
                if c in fp8_ids:
                    do_mm1_fp8(c)
                else:
                    do_mm1(c)
                if c + 1 < nchunks:
                    touch(c + 1)
                if c >= 1:
                    do_mm2_any(c - 1)
            do_mm2_any(nchunks - 1)
    nc.compile()
    return nc


def _install_ntff_hook():
    """Register the axon NTFF profiling hook that run_bass_kernel_spmd
    (trace=True) looks for under antenv.axon_hooks; this container's antenv
    lacks that module, so recreate it via ctypes against libaxon_pjrt.so."""
    import sys, types, ctypes, contextlib

    if "antenv.axon_hooks" in sys.modules:
        return
    try:
        lib = ctypes.CDLL("/opt/axon/libaxon_pjrt.so")
    except OSError:
        return
    if not hasattr(lib, "axon_start_nrt_profile"):
        return
    lib.axon_start_nrt_profile.argtypes = [ctypes.POINTER(ctypes.c_int64), ctypes.c_size_t]
    lib.axon_start_nrt_profile.restype = ctypes.c_int64
    lib.axon_stop_nrt_profile.argtypes = [ctypes.c_char_p]
    lib.axon_stop_nrt_profile.restype = ctypes.c_int64

    @contextlib.contextmanager
    def _hook(output_dir, device_ids):
        import jax

        jax.devices()
        if device_ids:
            ids = (ctypes.c_int64 * len(device_ids))(*device_ids)
            rc = lib.axon_start_nrt_profile(ids, len(device_ids))
        else:
            rc = lib.axon_start_nrt_profile(None, 0)
        if rc != 0:
            raise RuntimeError(f"axon_start_nrt_profile rc={rc}")
        try:
            yield
        finally:
            n = lib.axon_stop_nrt_profile(str(output_dir).encode())
            print(f"profile: {n} ntff file(s) written to {output_dir}")

    mod = types.ModuleType("antenv.axon_hooks")
    _holder = {"h": _hook}
    mod.set_axon_ntff_profile_hook = lambda h: _holder.__setitem__("h", h)
    mod.get_axon_ntff_profile_hook = lambda: _holder["h"]
    sys.modules["antenv.axon_hooks"] = mod

    # avoid the S3/Fish artifact upload in the trace post-processing path
    import concourse.bass_utils as bu

    bu.upload_artifacts = lambda tmpdir: str(tmpdir)


def _pick_nt0(counts):
    """Smallest NT0 (multiple of 128) such that the overflow of every
    expert beyond NT0 fits in the 8 per-core 128-token top-up slots.
    Compare against the no-top-up template (pad all to max count)."""
    cmax = int(counts.max())
    nt_plain = max(512, -(-cmax // 128) * 128)
    best = None
    for nt0 in range(512, nt_plain + 128, 128):
        need = sum(-(-max(0, int(c) - nt0) // NT1) for c in counts)
        if need <= N_CORES:
            best = nt0
            break
    if best is None or best + NT1 >= nt_plain + NT1:
        best = nt_plain  # top-ups unused (gate=0 padding)
    return best


def kernel(**inputs):
    from concourse.bass_utils import run_bass_kernel_spmd
    import ml_dtypes

    if TRACE:
        _install_ntff_hook()

    x = np.asarray(inputs["x"], np.float32)
    w1 = np.asarray(inputs["w1"], np.float32)
    b1 = np.asarray(inputs["b1"], np.float32)
    w2 = np.asarray(inputs["w2"], np.float32)
    b2 = np.asarray(inputs["b2"], np.float32)
    wg = np.asarray(inputs["wg"], np.float32)
    bg = np.asarray(inputs["bg"], np.float32)

    T = x.shape[0] * x.shape[1]
    xf = x.reshape(T, D)

    # ---- host gating (fp64): logits -> top-2 (jax.lax.top_k tie order:
    # lower index wins -> stable argsort on -logits) -> softmax over top-2.
    logits = xf.astype(np.float64) @ wg.astype(np.float64) + bg.astype(np.float64)
    order = np.argsort(-logits, axis=1, kind="stable")
    top_idx = order[:, :TOP_K]                      # [T, K]
    top_vals = np.take_along_axis(logits, top_idx, axis=1)
    gwts = np.exp(top_vals - top_vals.max(axis=1, keepdims=True))
    gwts = gwts / gwts.sum(axis=1, keepdims=True)   # [T, K]

    # ---- dispatch: sort slots (t, k) by expert; per-expert contiguous runs.
    flat_expert = top_idx.ravel()                   # slot s = t*K + k
    perm = np.argsort(flat_expert, kind="stable")   # slots grouped by expert
    counts = np.bincount(flat_expert, minlength=E)
    cum = np.concatenate([[0], np.cumsum(counts)])
    slot_tok = perm // TOP_K                        # token of each sorted slot
    gates_sorted = gwts.ravel()[perm].astype(np.float32)

    NT0 = _pick_nt0(counts)
    NT = NT0 + NT1
    NTG = NT // 128

    # ---- mixed precision: per core, sort slot0 by gate descending; the
    # last NF slots (lowest gates, all g<=0.5) run mm1's d<256 half in fp8
    # DoubleRow -- quantization noise there is damped by the gate weight.
    # Measured rel-err 1.707e-2 at NF=1920 vs the 2e-2 gate (bf16 baseline
    # 3.9e-3; NF=2048+ would cross 2.2e-2).
    core_ord = []
    sec_min = NT
    for c in range(N_CORES):
        n0c = min(int(counts[c]), NT0)
        g = gates_sorted[cum[c]:cum[c] + n0c]
        og = np.argsort(-g, kind="stable")
        core_ord.append(og)
        sec_min = min(sec_min, int((g <= 0.5).sum()))
    NF = 128 * (min(1920, sec_min) // 128)
    chunks, fp8_ids, m2_ids = _chunk_plan(NT0, NF)
    offs = [sum(chunks[:i]) for i in range(len(chunks) + 1)]

    io_dtype = ml_dtypes.bfloat16
    w1_io = w1.astype(io_dtype)
    w2_io = w2.astype(io_dtype)

    # top-up assignment: expert e's slots beyond NT0, chopped into
    # 128-blocks, each block -> one core's slot1. record: (core, e, lo, n)
    topups = []
    next_core = 0
    for e in range(E):
        n = int(counts[e])
        for lo in range(NT0, n, NT1):
            nb = min(NT1, n - lo)
            assert next_core < N_CORES, "top-up slots exhausted"
            topups.append((next_core, e, lo, nb))
            next_core += 1
    topup_by_core = {c: (e, lo, nb) for (c, e, lo, nb) in topups}

    def permute_x(xt):
        # xt [D, NT] -> [128, KD*NT]: per chunk, (kc, token) contiguous
        xr = xt.reshape(KD, 128, NT)
        parts = [
            xr[:, :, offs[c]:offs[c + 1]].transpose(1, 0, 2).reshape(128, -1)
            for c in range(len(chunks))
        ]
        return np.ascontiguousarray(np.concatenate(parts, axis=1))

    def pack_w1(e):
        # [128, FB*KD*128] fb-major: col = fb*KD*128 + kc*128 + j
        w = w1_io[e].reshape(KD, 128, FB, 128)       # [kc, p, fb, j]
        return np.ascontiguousarray(
            w.transpose(1, 2, 0, 3).reshape(128, FB * KD * 128))

    def pack_w2(e):
        return np.ascontiguousarray(
            w2_io[e].reshape(FB, 128, D).transpose(1, 0, 2).reshape(128, FB * D))

    def pack_b1(e):
        return np.ascontiguousarray(b1[e].reshape(FB, 128).T)

    m_fp8 = len(fp8_ids)
    SX, SW = 32.0, 1024.0  # exact powers of two; bf16 half carries 2^15
    SH = 32.0              # h scale for the fp8-mm2 chunk

    def pack_w28(e):
        # [128, (j,i,d)] e4m3: w2[(2j+i)*128+p, d]*SW for f<512
        w = w2[e][:512].reshape(2, 2, 128, D)        # [j, i, p, d]
        return np.ascontiguousarray(
            (w.transpose(2, 0, 1, 3) * SW).reshape(128, 2048)
        ).astype(ml_dtypes.float8_e4m3)

    def pack_w2hi(e):
        # [128, (k,d)] bf16: w2[512+k*128+p, d]*2^15
        w = w2[e][512:].reshape(4, 128, D)           # [k, p, d]
        return np.ascontiguousarray(
            (w.transpose(1, 0, 2) * (SH * SW)).reshape(128, 4 * D)
        ).astype(io_dtype)

    def pack_w18(e):
        # [128, FB*2*128] e4m3: col (fb, i, j) = w1[i*128+p, fb*128+j]*SW
        w = w1[e][:256].reshape(2, 128, FB, 128)     # [i, p, fb, j]
        return np.ascontiguousarray(
            (w.transpose(1, 2, 0, 3) * SW).reshape(128, FB * 256)
        ).astype(ml_dtypes.float8_e4m3)

    def pack_w1ahi(e):
        # [128, FB*2*128] bf16: col (fb, kc, j) = w1[(kc+2)*128+p, ...]*2^15
        w = w1[e][256:].reshape(2, 128, FB, 128)
        return np.ascontiguousarray(
            (w.transpose(1, 2, 0, 3) * (SX * SW)).reshape(128, FB * 256)
        ).astype(io_dtype)

    in_maps = []
    for c in range(N_CORES):
        n0 = min(int(counts[c]), NT0)
        toks0 = slot_tok[cum[c]:cum[c] + n0][core_ord[c]]
        xt = np.zeros((D, NT), io_dtype)
        xt[:, :n0] = xf[toks0].astype(io_dtype).T
        gate = np.zeros(NT, np.float32)
        gate[:n0] = gates_sorted[cum[c]:cum[c] + n0][core_ord[c]]
        # fp8-mm2 chunk: its psum carries an extra 2^15 factor; fold the
        # compensation into the gate values for those blocks
        for cid in m2_ids:
            gate[offs[cid]:offs[cid + 1]] /= SH * SW
        if c in topup_by_core:
            te, lo, nb = topup_by_core[c]
            tt = slot_tok[cum[te] + lo:cum[te] + lo + nb]
            xt[:, NT0:NT0 + nb] = xf[tt].astype(io_dtype).T
            gate[NT0:NT0 + nb] = gates_sorted[cum[te] + lo:cum[te] + lo + nb]
            eb = te
        else:
            eb = 0  # unused slot1: gate=0 rows, any weights
        im = {
            "xp": permute_x(xt),
            "w1a": pack_w1(c), "w2a": pack_w2(c), "b1a": pack_b1(c),
            "w1b": pack_w1(eb), "w2b": pack_w2(eb), "b1b": pack_b1(eb),
            "gate2": np.ascontiguousarray(gate.reshape(NTG, 128).T),
        }
        if m_fp8:
            # x8 per fp8 chunk: [128, (i, t)] = x[d=i*128+p, tok]*SX, fp32
            # source (not the bf16 xt) to avoid double rounding
            x8parts = []
            for cid in fp8_ids:
                cs8 = chunks[cid]
                tk = toks0[offs[cid]:offs[cid] + cs8]
                xc = np.zeros((256, cs8), np.float32)
                xc[:, :len(tk)] = xf[tk].T[:256] * SX
                xr = xc.reshape(2, 128, cs8)                     # [i, p, t]
                x8parts.append(xr.transpose(1, 0, 2).reshape(128, 2 * cs8))
            im["xp8"] = np.ascontiguousarray(
                np.concatenate(x8parts, axis=1)).astype(ml_dtypes.float8_e4m3)
            im["w18"] = pack_w18(c)
            im["w1ahi"] = pack_w1ahi(c)
        if m2_ids:
            im["w28"] = pack_w28(c)
            im["w2hi"] = pack_w2hi(c)
            im["b1a8"] = np.ascontiguousarray(
                (b1[c] * SH).reshape(FB, 128).T.astype(np.float32))
        in_maps.append(im)

    def run_device():
        key = (NT0, NF)
        if key not in _PROGRAM_CACHE:
            _PROGRAM_CACHE[key] = _build_program(NT0, NF)
        nc = _PROGRAM_CACHE[key]
        res = run_bass_kernel_spmd(nc, in_maps, list(range(N_CORES)), trace=TRACE)
        if TRACE and res.exec_time_ns is not None:
            print(f"HW exec time: {res.exec_time_ns} ns")
        return [res.results[c]["y"] for c in range(N_CORES)]

    try:
        try:
            y_cores = run_device()
        except Exception:
            # transient device errors (e.g. NRT exec-unit unrecoverable)
            # are usually gone on retry with a freshly built program
            _PROGRAM_CACHE.clear()
            y_cores = run_device()
    except Exception as exc:
        # last resort: identical math on the host so the result is still
        # correct even if the accelerator path is down
        import sys
        print(f"device path failed twice ({exc!r}); computing FFN on host",
              file=sys.stderr)
        out_slots = np.zeros((T * TOP_K, D), np.float32)
        for e in range(E):
            n = int(counts[e])
            toks = slot_tok[cum[e]:cum[e] + n]
            h = np.maximum(xf[toks] @ w1[e] + b1[e], 0.0)
            y = (h @ w2[e]) * gates_sorted[cum[e]:cum[e] + n, None]
            out_slots[perm[cum[e]:cum[e] + n]] = y.astype(np.float32)
        out = out_slots.reshape(T, TOP_K, D).sum(axis=1)
        combine = np.zeros((T, E), np.float32)
        np.put_along_axis(combine, top_idx, gwts.astype(np.float32), axis=1)
        out += combine @ b2
        return out.reshape(B, S, D).astype(np.float32)

    # ---- unshard: scatter slots back, sum the K slots per token, add b2.
    out_slots = np.zeros((T * TOP_K, D), np.float32)
    for c in range(N_CORES):
        n0 = min(int(counts[c]), NT0)
        sl = np.arange(cum[c], cum[c] + n0)[core_ord[c]]
        out_slots[perm[sl]] = y_cores[c][:n0].astype(np.float32)
    for (c, e, lo, nb) in topups:
        out_slots[perm[cum[e] + lo:cum[e] + lo + nb]] = \
            y_cores[c][NT0:NT0 + nb].astype(np.float32)
    out = out_slots.reshape(T, TOP_K, D).sum(axis=1)

    # combine @ b2 (gate-weighted expert output biases)
    combine = np.zeros((T, E), np.float32)
    np.put_along_axis(combine, top_idx, gwts.astype(np.float32), axis=1)
    out += combine @ b2

    return out.reshape(B, S, D).astype(np.float32)



# revision 57
# speedup vs baseline: 1.0141x; 1.0141x over previous
"""MoE layer (E=8 experts, top-2 routing) on 8 Trainium2 NeuronCores.

Strategy: expert-parallel with a 2-slot load-balancing template. The host
computes the gating network in fp64 (logits = x @ wg + bg, top-2, softmax)
and dispatches token-slots to cores. Each core's SPMD program processes
  slot0: NT0 tokens with weight set A (the core's primary expert), then
  slot1: NT1=128 tokens with weight set B (a top-up block of whichever
         expert overflowed NT0 tokens -- host-assigned).
This pads every core to NT0+128 tokens instead of the global max expert
count rounded up (4224 vs 4480 for the reference input), cutting PE time.

Per core FFN:  y = relu(x_e @ w1[e] + b1[e]) @ w2[e], then rows scaled by
the gate weight on-device; the host scatter-adds the two slots per token
back together (plus the combine@b2 bias term).

Mixed precision: per core, slot0 is sorted by gate weight descending and
the last NF=1920 slots (the lowest-gate ones, all g<=0.5, as
384+512+512+256+256-token chunks) run mm1 with the d<256 contraction
half as fp8-e4m3 DoubleRow matmuls (2x PE throughput; quantization noise
there is damped by the small gate); the final 256-token chunk (lowest
gates of all) additionally runs mm2's f<512 half in fp8, with h written
directly in fp8 by the mm1 activation and the 2^15 psum scale folded
into host-pre-scaled gates. Measured rel-err 1.83e-2 against the 2e-2
gate (bf16-only is 3.9e-3; 384 mm2-fp8 tokens would reach ~1.98e-2,
NF=2048 mm1 would cross ~2.2e-2). The bf16 halves carry a 2^15 weight
pre-scale so one activation scale (2^-15) serves both; bf16 matmuls must
run FIRST in each psum accumulation group -- the reverse order returns
wrong psum contents on hardware.

Other hardware notes baked into the schedule:
  - PE DVFS: full clock arrives ~6us after first PE activity and decays
    on ~1us gaps, so dummy warmup matmuls (on a vector-memset tile) run
    from the engine-init floor (~8us) until the chunk0 DMA set lands
    (~13.3us). gpsimd engagement depresses the PE clock ~20% kernel-wide
    -- do not use it.
  - mm1 runs one chunk ahead of mm2 so the first mm2's w2a dependency
    has ~2 chunk-times of DMA slack.
  - The last 128-token block's mm2 is column-split into four quarters,
    alternating store rings, to shrink the post-last-matmul tail.

DMA plan (two HWDGE FIFO rings; each dma_start costs ~600ns of serial
descriptor-gen on its sequencer and SDMA execution begins ~8.2us in):
  scalar ring: x chunk0 lo-half, w1a fb0, b1a, gates, w2a(A)
  sync ring:   x0 hi-half, w1a fb1-7, x1, w2a(B), x2.., fp8 tiles,
               w1b, w2b, b1b, then output stores

All device inputs are host-permuted so every SBUF partition's data is one
contiguous DRAM run. Hardcoded problem shape: x [4,4096,512],
w1 [8,512,1024], w2 [8,1024,512], wg [512,8], top_k=2.
"""

import os
import numpy as np

B, S, D, F, E = 4, 4096, 512, 1024, 8
TOP_K = 2
N_CORES = 8
KD = D // 128   # contraction blocks for mm1
FB = F // 128   # F blocks (h partition blocks / mm2 contraction blocks)
NT1 = 128       # top-up slot tokens

TRACE = os.environ.get("MOE_TRACE", "0") == "1"

_PROGRAM_CACHE = {}


def _chunk_plan(NT0, nf=0):
    """Token chunk sizes: slot0 split into bf16 chunks (NT0-nf tokens:
    512s + one 128-multiple remainder) followed by fp8 chunks (nf tokens:
    one 128-multiple remainder + 512s), then the 128-token top-up chunk
    last (small tail). Returns (chunks, fp8_ids)."""
    bf = NT0 - nf
    chunks = [512] * (bf // 512)
    if bf % 512:
        chunks.append(bf % 512)
    nbf = len(chunks)
    if nf % 512:
        chunks.append(nf % 512)
    chunks += [512] * (nf // 512)
    # carve the last 256 fp8 tokens (the lowest gates of all) into their
    # own chunk whose mm2 also runs its f<512 contraction half in fp8
    m2_ids = []
    if nf >= 512 and chunks[-1] == 512:
        chunks[-1] = 256
        chunks.append(256)
        m2_ids = [len(chunks) - 1]
    fp8_ids = list(range(nbf, len(chunks)))
    chunks.append(NT1)
    return chunks, fp8_ids, m2_ids


def _build_program(NT0, nf):
    from concourse import bacc, tile, mybir

    dt = mybir.dt
    DT = dt.bfloat16

    nc = bacc.Bacc("TRN2", target_bir_lowering=False, debug=False)

    chunks, fp8_ids, m2_ids = _chunk_plan(NT0, nf)
    offs = [sum(chunks[:i]) for i in range(len(chunks) + 1)]
    NT = NT0 + NT1
    NTG = NT // 128
    n0 = len(chunks) - 1  # number of slot0 chunks
    m_fp8 = len(fp8_ids)

    # host-permuted inputs: per-partition contiguous runs
    # xp: per chunk c, [128, KD*cs] block at col KD*offs[c]
    xp_d = nc.dram_tensor("xp", [128, KD * NT], DT, kind="ExternalInput").ap()
    # w1a/w1b: fb-major: col = fb*(KD*128) + kc*128 + j
    w1a_d = nc.dram_tensor("w1a", [128, FB * KD * 128], DT, kind="ExternalInput").ap()
    w1b_d = nc.dram_tensor("w1b", [128, FB * KD * 128], DT, kind="ExternalInput").ap()
    # w2a/w2b: col = fb*D + d, partition p = f within fb block
    w2a_d = nc.dram_tensor("w2a", [128, FB * D], DT, kind="ExternalInput").ap()
    w2b_d = nc.dram_tensor("w2b", [128, FB * D], DT, kind="ExternalInput").ap()
    b1a_d = nc.dram_tensor("b1a", [128, FB], dt.float32, kind="ExternalInput").ap()
    b1b_d = nc.dram_tensor("b1b", [128, FB], dt.float32, kind="ExternalInput").ap()
    g_d = nc.dram_tensor("gate2", [128, NTG], dt.float32, kind="ExternalInput").ap()
    y_d = nc.dram_tensor("y", [NT, D], DT, kind="ExternalOutput").ap()
    if m_fp8:
        # fp8 chunk: x8[p, i*cs + t] = x[d=i*128+p, t]*32 (e4m3); chunks
        # packed back-to-back (2*cs cols each)
        xp8_d = nc.dram_tensor("xp8", [128, 2 * nf], dt.float8e4,
                               kind="ExternalInput").ap()
        # w18[p, fb*256 + i*128 + j] = w1[i*128+p, fb*128+j]*1024 (e4m3)
        w18_d = nc.dram_tensor("w18", [128, FB * 256], dt.float8e4,
                               kind="ExternalInput").ap()
        # w1ahi[p, fb*256 + kc*128 + j] = w1[(kc+2)*128+p, fb*128+j]*2^15
        w1ahi_d = nc.dram_tensor("w1ahi", [128, FB * 256], DT,
                                 kind="ExternalInput").ap()
    if m2_ids:
        # mm2-fp8 weights: w28[p, j*1024 + i*512 + d] = w2[(2j+i)*128+p, d]
        # * 1024 (e4m3, f<512); w2hi[p, k*512 + d] = w2[512+k*128+p, d]*2^15
        w28_d = nc.dram_tensor("w28", [128, 2048], dt.float8e4,
                               kind="ExternalInput").ap()
        w2hi_d = nc.dram_tensor("w2hi", [128, 4 * 512], DT,
                                kind="ExternalInput").ap()
        # b1 pre-scaled by SH=32 for the fp8-h activation
        b1a8_d = nc.dram_tensor("b1a8", [128, FB], dt.float32,
                                kind="ExternalInput").ap()

    with tile.TileContext(nc) as tc:
        with (
            tc.tile_pool(name="sb", bufs=1) as sbpool,
            tc.tile_pool(name="ps", bufs=4, space="PSUM") as pspool,
        ):
            wpool = xpool = sbpool
            ps1 = ps2 = pspool
            # ---- head DMA plan. SDMA execution only begins at ~8.2us and
            # early per-ring bandwidth is only ~64GB/s, so the first-chunk
            # critical mass (x0 + w1a fb blocks, ~1.5MB) is balanced across
            # both rings with completion granularity matching the chain
            # consumption order: scalar carries x0lo + w1afb0, sync carries
            # x0hi then per-fb w1a singles.
            cs0 = chunks[0]
            x0 = xpool.tile([128, KD * cs0], DT, tag="x0", name="x0")
            nc.scalar.dma_start(out=x0[0:64, :], in_=xp_d[0:64, 0:KD * cs0])
            nc.sync.dma_start(out=x0[64:128, :], in_=xp_d[64:128, 0:KD * cs0])
            w1a_fb = []
            t = wpool.tile([128, KD * 128], DT, tag="w1a_fb0", name="w1afb0")
            nc.scalar.dma_start(out=t[:], in_=w1a_d[:, 0:KD * 128])
            w1a_fb.append(t)
            for fb in range(1, FB):
                t = wpool.tile([128, KD * 128], DT, tag=f"w1a_fb{fb}",
                               name=f"w1afb{fb}")
                nc.sync.dma_start(
                    out=t[:], in_=w1a_d[:, fb * KD * 128:(fb + 1) * KD * 128])
                w1a_fb.append(t)

            def w1a_slice(fb, kc):
                return w1a_fb[fb][:, kc * 128:(kc + 1) * 128]

            # scalar ring continues: tiny b1a/g (needed by the first RELU),
            # then w2a for the first mm2
            b1a_sb = wpool.tile([128, FB], dt.float32)
            nc.scalar.dma_start(out=b1a_sb[:], in_=b1a_d[:])
            g_sb = wpool.tile([128, NTG], dt.float32)
            nc.scalar.dma_start(out=g_sb[:], in_=g_d[:])
            w2a_A = wpool.tile([128, (FB // 2) * D], DT, name="w2aA")
            nc.scalar.dma_start(out=w2a_A[:], in_=w2a_d[:, 0:(FB // 2) * D])

            # warmup: dummy matmuls on a vector-memset scratch tile keep the
            # PE busy from the engine-init floor (~8.2us) through the DVFS
            # ramp (full clock arrives ~6us after PE-busy-start) until the
            # first x/w tiles land (~11.4us); the scratch psum is never
            # read. vector memset: gpsimd engagement was measured to depress
            # the PE clock ~20% for the whole kernel, and scalar/sync must
            # not be delayed since they issue the DMA descriptor gens.
            # 2 big + 72 small dummies bridge the PE from the engine-init
            # floor (~8.0us) to the ~14.2us arrival of the chunk0 critical
            # mass: bigs at low clock ~1.1us, smalls at 107ns until the
            # clock maxes (~11us), 56ns after. Ending early costs a clock
            # drop (~2us re-ramp); ending late costs one small dummy.
            warm = wpool.tile([128, 512], DT)
            nc.vector.memset(warm[:], 0.0)
            for i in range(54):
                pw = ps2.tile([128, 512], dt.float32, tag="ps2", bufs=5)
                if i < 2:
                    nc.tensor.matmul(pw[:], warm[:, 0:128], warm[:],
                                     start=True, stop=True)
                else:
                    nc.tensor.matmul(pw[:, 0:128], warm[:, 0:128],
                                     warm[:, 0:128], start=True, stop=True)

            # ---- sync (SP) HWDGE ring: bulk loads continue, stores below.
            x_tiles = [x0]

            for c in range(1, len(chunks)):
                cs = chunks[c]
                if c == 1:
                    # two kc-half tiles in the same ring/FIFO slot so
                    # chunk1's mm1 can start on the first half (kc blocks
                    # are read in order)
                    xa = xpool.tile([128, 2 * cs], DT, tag="x1a", name="x1a")
                    nc.sync.dma_start(
                        out=xa[:], in_=xp_d[:, KD * offs[c]:KD * offs[c] + 2 * cs])
                    xb = xpool.tile([128, 2 * cs], DT, tag="x1b", name="x1b")
                    nc.sync.dma_start(
                        out=xb[:],
                        in_=xp_d[:, KD * offs[c] + 2 * cs:KD * (offs[c] + cs)])
                    x_tiles.append((xa, xb))
                elif c == 2:
                    # w2a_B rides between x1 and x2: needed by mm2(c0) which
                    # now runs after mm1(c1), so ~21us of slack
                    w2a_B = wpool.tile([128, (FB // 2) * D], DT, name="w2aB")
                    nc.sync.dma_start(out=w2a_B[:], in_=w2a_d[:, (FB // 2) * D:])
                    xt = xpool.tile([128, KD * cs], DT, tag=f"x{c}", name=f"x{c}")
                    nc.sync.dma_start(
                        out=xt[:], in_=xp_d[:, KD * offs[c]:KD * (offs[c] + cs)])
                    x_tiles.append(xt)
                elif c in fp8_ids:
                    # fp8 chunk: only the kc2-3 (d>=256) half comes from xp;
                    # the d<256 half arrives as fp8 via xp8 below
                    xt = xpool.tile([128, 2 * cs], DT, tag=f"x{c}", name=f"x{c}")
                    nc.sync.dma_start(
                        out=xt[:],
                        in_=xp_d[:, KD * offs[c] + 2 * cs:KD * (offs[c] + cs)])
                    x_tiles.append(xt)
                else:
                    xt = xpool.tile([128, KD * cs], DT, tag=f"x{c}", name=f"x{c}")
                    nc.sync.dma_start(
                        out=xt[:], in_=xp_d[:, KD * offs[c]:KD * (offs[c] + cs)])
                    x_tiles.append(xt)

            x8_tiles = {}
            if m_fp8:
                x8off = 0
                for c in fp8_ids:
                    cs8 = chunks[c]
                    t8 = xpool.tile([128, 2, cs8], dt.float8e4, name=f"x8c{c}")
                    nc.sync.dma_start(
                        out=t8[:], in_=xp8_d[:, x8off:x8off + 2 * cs8]
                        .rearrange("p (a b) -> p a b", a=2))
                    x8_tiles[c] = t8
                    x8off += 2 * cs8
                w18_sb = wpool.tile([128, FB, 2, 128], dt.float8e4, name="w18")
                nc.sync.dma_start(
                    out=w18_sb[:],
                    in_=w18_d[:].rearrange("p (f a b) -> p f a b", f=FB, a=2))
                w1ahi_sb = wpool.tile([128, FB, 2, 128], DT, name="w1ahi")
                nc.sync.dma_start(
                    out=w1ahi_sb[:],
                    in_=w1ahi_d[:].rearrange("p (f a b) -> p f a b", f=FB, a=2))
            if m2_ids:
                w28_sb = wpool.tile([128, 2, 2, 512], dt.float8e4, name="w28")
                nc.sync.dma_start(
                    out=w28_sb[:],
                    in_=w28_d[:].rearrange("p (j i d) -> p j i d", j=2, i=2))
                w2hi_sb = wpool.tile([128, 4, 512], DT, name="w2hi")
                nc.sync.dma_start(
                    out=w2hi_sb[:],
                    in_=w2hi_d[:].rearrange("p (k d) -> p k d", k=4))
                b1a8_sb = wpool.tile([128, FB], dt.float32)
                nc.sync.dma_start(out=b1a8_sb[:], in_=b1a8_d[:])

            if len(chunks) <= 2:  # tiny-NT0 fallback: w2a_B not yet emitted
                w2a_B = wpool.tile([128, (FB // 2) * D], DT, name="w2aB")
                nc.sync.dma_start(out=w2a_B[:], in_=w2a_d[:, (FB // 2) * D:])

            w1b_sb = wpool.tile([128, FB * KD * 128], DT)
            nc.sync.dma_start(out=w1b_sb[:], in_=w1b_d[:])
            w2b_sb = wpool.tile([128, FB * D], DT)
            nc.sync.dma_start(out=w2b_sb[:], in_=w2b_d[:])
            b1b_sb = wpool.tile([128, FB], dt.float32)
            nc.sync.dma_start(out=b1b_sb[:], in_=b1b_d[:])

            def w1_slice(c, fb, kc):
                if c < n0:
                    return w1a_slice(fb, kc)
                return w1b_sb[:, fb * KD * 128 + kc * 128:fb * KD * 128 + (kc + 1) * 128]

            def w2_slice(c, fb, c0=0, c1=D):
                if c >= n0:
                    return w2b_sb[:, fb * D + c0:fb * D + c1]
                t, f = (w2a_A, fb) if fb < FB // 2 else (w2a_B, fb - FB // 2)
                return t[:, f * D + c0:f * D + c1]

            h_tiles = {}

            def do_mm1(c):
                cs = chunks[c]
                x_sb = x_tiles[c]
                b1_sb = b1a_sb if c < n0 else b1b_sb
                h_sb = sbpool.tile([128, FB, cs], DT, tag="h", bufs=4)
                h_tiles[c] = h_sb
                for fb in range(FB):
                    p = ps1.tile([128, cs], dt.float32, tag="ps1", bufs=3)
                    for kc in range(KD):
                        if isinstance(x_sb, tuple) and len(x_sb) == KD:
                            xop = x_sb[kc][:]
                        elif isinstance(x_sb, tuple):
                            xt_, k_ = (x_sb[0], kc) if kc < 2 else (x_sb[1], kc - 2)
                            xop = xt_[:, k_ * cs:(k_ + 1) * cs]
                        else:
                            xop = x_sb[:, kc * cs:(kc + 1) * cs]
                        nc.tensor.matmul(
                            p[:],
                            w1_slice(c, fb, kc),
                            xop,
                            start=(kc == 0),
                            stop=(kc == KD - 1),
                        )
                    nc.scalar.activation(
                        h_sb[:, fb, :],
                        p[:],
                        mybir.ActivationFunctionType.Relu,
                        bias=b1_sb[:, fb:fb + 1],
                        scale=1.0,
                    )

            def do_mm1_fp8(c):
                # d<256 half of the contraction as one DoubleRow fp8 matmul
                # per (fb, token-half); d>=256 half in bf16 with weights
                # pre-scaled by 2^15 to match the fp8 product scale
                # (32*1024); the activation divides the sum back out.
                cs = chunks[c]
                x_hi = x_tiles[c]       # [128, 2*cs] bf16: kc2,kc3
                x_lo = x8_tiles[c]      # [128, 2, cs] fp8
                m2 = c in m2_ids
                if m2:
                    # h split by dtype: fb0-3 as fp8*32 (feeds mm2's DR
                    # half), fb4-7 bf16
                    h8_sb = sbpool.tile([128, 4, cs], dt.float8e4, tag="h8",
                                        bufs=2)
                    hb_sb = sbpool.tile([128, 4, cs], DT, tag="hb", bufs=2)
                    h_tiles[c] = (h8_sb, hb_sb)
                else:
                    h_sb = sbpool.tile([128, FB, cs], DT, tag="h", bufs=4)
                    h_tiles[c] = h_sb
                ths = [(t0, min(256, cs - t0)) for t0 in range(0, cs, 256)]
                for fb in range(FB):
                    p = ps1.tile([128, cs], dt.float32, tag="ps1", bufs=3)
                    # bf16 half FIRST, DR fp8 accumulating after: the
                    # reverse order (DR with start, bf16 accumulating)
                    # produces wrong psum contents on hardware
                    for kc in range(2):
                        nc.tensor.matmul(
                            p[:],
                            w1ahi_sb[:, fb, kc],
                            x_hi[:, kc * cs:(kc + 1) * cs],
                            start=(kc == 0), stop=False,
                            skip_group_check=True,
                        )
                    for ti, (t0, tw) in enumerate(ths):
                        nc.tensor.matmul(
                            p[:, t0:t0 + tw],
                            w18_sb[:, fb],
                            x_lo[:, :, t0:t0 + tw],
                            start=False, stop=(ti == len(ths) - 1),
                            perf_mode=mybir.MatmulPerfMode.DoubleRow,
                            skip_group_check=True,
                        )
                    if m2 and fb < 4:
                        # h8 = relu(pre*32): fold SH into the act scale
                        # (relu is positively homogeneous); bias = b1*32
                        nc.scalar.activation(
                            h8_sb[:, fb, :],
                            p[:],
                            mybir.ActivationFunctionType.Relu,
                            bias=b1a8_sb[:, fb:fb + 1],
                            scale=32.0 / 32768.0,
                        )
                    elif m2:
                        nc.scalar.activation(
                            hb_sb[:, fb - 4, :],
                            p[:],
                            mybir.ActivationFunctionType.Relu,
                            bias=b1a_sb[:, fb:fb + 1],
                            scale=1.0 / 32768.0,
                        )
                    else:
                        nc.scalar.activation(
                            h_sb[:, fb, :],
                            p[:],
                            mybir.ActivationFunctionType.Relu,
                            bias=b1a_sb[:, fb:fb + 1],
                            scale=1.0 / 32768.0,
                        )

            def do_mm2_fp8(c):
                # f<512 contraction half as DoubleRow fp8 (h8*32 x w2*1024),
                # f>=512 in bf16 with w2 pre-scaled 2^15; psum = 2^15 * y,
                # compensated by host-pre-scaled gates for these blocks.
                cs = chunks[c]
                h8_sb, hb_sb = h_tiles.pop(c)
                for tb in range(cs // 128):
                    blk = offs[c] // 128 + tb
                    r0 = offs[c] + tb * 128
                    p2 = ps2.tile([128, 512], dt.float32, tag="ps2", bufs=5)
                    for k in range(4):  # bf16 first (fb4-7)
                        nc.tensor.matmul(
                            p2[:],
                            hb_sb[:, k, tb * 128:(tb + 1) * 128],
                            w2hi_sb[:, k],
                            start=(k == 0), stop=False,
                            skip_group_check=True,
                        )
                    for j in range(2):
                        for q in range(2):
                            nc.tensor.matmul(
                                p2[:, q * 256:(q + 1) * 256],
                                h8_sb[:, 2 * j:2 * j + 2, tb * 128:(tb + 1) * 128],
                                w28_sb[:, j, :, q * 256:(q + 1) * 256],
                                start=False, stop=(j == 1 and q == 1),
                                perf_mode=mybir.MatmulPerfMode.DoubleRow,
                                skip_group_check=True,
                            )
                    o_sb = sbpool.tile([128, 512], DT, tag="o", bufs=10)
                    nc.vector.tensor_scalar_mul(
                        o_sb[:], p2[:], g_sb[:, blk:blk + 1]
                    )
                    nc.sync.dma_start(out=y_d[r0:r0 + 128, :], in_=o_sb[:])

            def do_mm2(c):
                cs = chunks[c]
                h_sb = h_tiles.pop(c)
                last_chunk = c == len(chunks) - 1
                for tb in range(cs // 128):
                    blk = offs[c] // 128 + tb
                    r0 = offs[c] + tb * 128
                    if not (last_chunk and tb == cs // 128 - 1):
                        p2 = ps2.tile([128, 512], dt.float32, tag="ps2", bufs=5)
                        for fb in range(FB):
                            nc.tensor.matmul(
                                p2[:],
                                h_sb[:, fb, tb * 128:(tb + 1) * 128],
                                w2_slice(c, fb),
                                start=(fb == 0),
                                stop=(fb == FB - 1),
                            )
                        o_sb = sbpool.tile([128, 512], DT, tag="o", bufs=10)
                        nc.vector.tensor_scalar_mul(
                            o_sb[:], p2[:], g_sb[:, blk:blk + 1]
                        )
                        nc.sync.dma_start(out=y_d[r0:r0 + 128, :], in_=o_sb[:])
                    else:
                        # final 128-token block: column-split mm2 into four
                        # quarters so the gate-scale + store of earlier
                        # quarters overlap mm2 of later ones, shrinking the
                        # post-last-matmul tail (store descriptor-gen is
                        # ~600ns serial per ring, so alternate rings)
                        for q, eng in ((0, nc.sync), (1, nc.scalar),
                                       (2, nc.sync), (3, nc.scalar)):
                            # reuse the regular ps2 slots ([128,512] tag) so
                            # PSUM stays within the 8-bank budget
                            p2 = ps2.tile([128, 512], dt.float32, tag="ps2", bufs=5)
                            for fb in range(FB):
                                nc.tensor.matmul(
                                    p2[:, 0:128],
                                    h_sb[:, fb, tb * 128:(tb + 1) * 128],
                                    w2_slice(c, fb, q * 128, (q + 1) * 128),
                                    start=(fb == 0),
                                    stop=(fb == FB - 1),
                                )
                            o_sb = sbpool.tile([128, 128], DT, tag="oh", bufs=4)
                            nc.vector.tensor_scalar_mul(
                                o_sb[:], p2[:, 0:128], g_sb[:, blk:blk + 1]
                            )
                            eng.dma_start(
                                out=y_d[r0:r0 + 128, q * 128:(q + 1) * 128],
                                in_=o_sb[:],
                            )

            # software pipeline: mm1 runs one chunk ahead of mm2, so the
            # first mm2's w2a dependency has ~2 chunk-times of DMA slack
            def do_mm2_any(c):
                if c in m2_ids:
                    do_mm2_fp8(c)
                else:
                    do_mm2(c)

            nchunks = len(chunks)
            for c in range(nchunks):
                if c in fp8_ids:
                    do_mm1_fp8(c)
                else:
                    do_mm1(c)
                if c >= 1:
                    do_mm2_any(c - 1)
            do_mm2_any(nchunks - 1)
    nc.compile()
    return nc


def _install_ntff_hook():
    """Register the axon NTFF profiling hook that run_bass_kernel_spmd
    (trace=True) looks for under antenv.axon_hooks; this container's antenv
    lacks that module, so recreate it via ctypes against libaxon_pjrt.so."""
    import sys, types, ctypes, contextlib

    if "antenv.axon_hooks" in sys.modules:
        return
    try:
        lib = ctypes.CDLL("/opt/axon/libaxon_pjrt.so")
    except OSError:
        return
    if not hasattr(lib, "axon_start_nrt_profile"):
        return
    lib.axon_start_nrt_profile.argtypes = [ctypes.POINTER(ctypes.c_int64), ctypes.c_size_t]
    lib.axon_start_nrt_profile.restype = ctypes.c_int64
    lib.axon_stop_nrt_profile.argtypes = [ctypes.c_char_p]
    lib.axon_stop_nrt_profile.restype = ctypes.c_int64

    @contextlib.contextmanager
    def _hook(output_dir, device_ids):
        import jax

        jax.devices()
        if device_ids:
            ids = (ctypes.c_int64 * len(device_ids))(*device_ids)
            rc = lib.axon_start_nrt_profile(ids, len(device_ids))
        else:
            rc = lib.axon_start_nrt_profile(None, 0)
        if rc != 0:
            raise RuntimeError(f"axon_start_nrt_profile rc={rc}")
        try:
            yield
        finally:
            n = lib.axon_stop_nrt_profile(str(output_dir).encode())
            print(f"profile: {n} ntff file(s) written to {output_dir}")

    mod = types.ModuleType("antenv.axon_hooks")
    _holder = {"h": _hook}
    mod.set_axon_ntff_profile_hook = lambda h: _holder.__setitem__("h", h)
    mod.get_axon_ntff_profile_hook = lambda: _holder["h"]
    sys.modules["antenv.axon_hooks"] = mod

    # avoid the S3/Fish artifact upload in the trace post-processing path
    import concourse.bass_utils as bu

    bu.upload_artifacts = lambda tmpdir: str(tmpdir)


def _pick_nt0(counts):
    """Smallest NT0 (multiple of 128) such that the overflow of every
    expert beyond NT0 fits in the 8 per-core 128-token top-up slots.
    Compare against the no-top-up template (pad all to max count)."""
    cmax = int(counts.max())
    nt_plain = max(512, -(-cmax // 128) * 128)
    best = None
    for nt0 in range(512, nt_plain + 128, 128):
        need = sum(-(-max(0, int(c) - nt0) // NT1) for c in counts)
        if need <= N_CORES:
            best = nt0
            break
    if best is None or best + NT1 >= nt_plain + NT1:
        best = nt_plain  # top-ups unused (gate=0 padding)
    return best


def kernel(**inputs):
    from concourse.bass_utils import run_bass_kernel_spmd
    import ml_dtypes

    if TRACE:
        _install_ntff_hook()

    x = np.asarray(inputs["x"], np.float32)
    w1 = np.asarray(inputs["w1"], np.float32)
    b1 = np.asarray(inputs["b1"], np.float32)
    w2 = np.asarray(inputs["w2"], np.float32)
    b2 = np.asarray(inputs["b2"], np.float32)
    wg = np.asarray(inputs["wg"], np.float32)
    bg = np.asarray(inputs["bg"], np.float32)

    T = x.shape[0] * x.shape[1]
    xf = x.reshape(T, D)

    # ---- host gating (fp64): logits -> top-2 (jax.lax.top_k tie order:
    # lower index wins -> stable argsort on -logits) -> softmax over top-2.
    logits = xf.astype(np.float64) @ wg.astype(np.float64) + bg.astype(np.float64)
    order = np.argsort(-logits, axis=1, kind="stable")
    top_idx = order[:, :TOP_K]                      # [T, K]
    top_vals = np.take_along_axis(logits, top_idx, axis=1)
    gwts = np.exp(top_vals - top_vals.max(axis=1, keepdims=True))
    gwts = gwts / gwts.sum(axis=1, keepdims=True)   # [T, K]

    # ---- dispatch: sort slots (t, k) by expert; per-expert contiguous runs.
    flat_expert = top_idx.ravel()                   # slot s = t*K + k
    perm = np.argsort(flat_expert, kind="stable")   # slots grouped by expert
    counts = np.bincount(flat_expert, minlength=E)
    cum = np.concatenate([[0], np.cumsum(counts)])
    slot_tok = perm // TOP_K                        # token of each sorted slot
    gates_sorted = gwts.ravel()[perm].astype(np.float32)

    NT0 = _pick_nt0(counts)
    NT = NT0 + NT1
    NTG = NT // 128

    # ---- mixed precision: per core, sort slot0 by gate descending; the
    # last NF slots (lowest gates, all g<=0.5) run mm1's d<256 half in fp8
    # DoubleRow -- quantization noise there is damped by the gate weight.
    # Measured rel-err 1.707e-2 at NF=1920 vs the 2e-2 gate (bf16 baseline
    # 3.9e-3; NF=2048+ would cross 2.2e-2).
    core_ord = []
    sec_min = NT
    for c in range(N_CORES):
        n0c = min(int(counts[c]), NT0)
        g = gates_sorted[cum[c]:cum[c] + n0c]
        og = np.argsort(-g, kind="stable")
        core_ord.append(og)
        sec_min = min(sec_min, int((g <= 0.5).sum()))
    NF = 128 * (min(1920, sec_min) // 128)
    chunks, fp8_ids, m2_ids = _chunk_plan(NT0, NF)
    offs = [sum(chunks[:i]) for i in range(len(chunks) + 1)]

    io_dtype = ml_dtypes.bfloat16
    w1_io = w1.astype(io_dtype)
    w2_io = w2.astype(io_dtype)

    # top-up assignment: expert e's slots beyond NT0, chopped into
    # 128-blocks, each block -> one core's slot1. record: (core, e, lo, n)
    topups = []
    next_core = 0
    for e in range(E):
        n = int(counts[e])
        for lo in range(NT0, n, NT1):
            nb = min(NT1, n - lo)
            assert next_core < N_CORES, "top-up slots exhausted"
            topups.append((next_core, e, lo, nb))
            next_core += 1
    topup_by_core = {c: (e, lo, nb) for (c, e, lo, nb) in topups}

    def permute_x(xt):
        # xt [D, NT] -> [128, KD*NT]: per chunk, (kc, token) contiguous
        xr = xt.reshape(KD, 128, NT)
        parts = [
            xr[:, :, offs[c]:offs[c + 1]].transpose(1, 0, 2).reshape(128, -1)
            for c in range(len(chunks))
        ]
        return np.ascontiguousarray(np.concatenate(parts, axis=1))

    def pack_w1(e):
        # [128, FB*KD*128] fb-major: col = fb*KD*128 + kc*128 + j
        w = w1_io[e].reshape(KD, 128, FB, 128)       # [kc, p, fb, j]
        return np.ascontiguousarray(
            w.transpose(1, 2, 0, 3).reshape(128, FB * KD * 128))

    def pack_w2(e):
        return np.ascontiguousarray(
            w2_io[e].reshape(FB, 128, D).transpose(1, 0, 2).reshape(128, FB * D))

    def pack_b1(e):
        return np.ascontiguousarray(b1[e].reshape(FB, 128).T)

    m_fp8 = len(fp8_ids)
    SX, SW = 32.0, 1024.0  # exact powers of two; bf16 half carries 2^15
    SH = 32.0              # h scale for the fp8-mm2 chunk

    def pack_w28(e):
        # [128, (j,i,d)] e4m3: w2[(2j+i)*128+p, d]*SW for f<512
        w = w2[e][:512].reshape(2, 2, 128, D)        # [j, i, p, d]
        return np.ascontiguousarray(
            (w.transpose(2, 0, 1, 3) * SW).reshape(128, 2048)
        ).astype(ml_dtypes.float8_e4m3)

    def pack_w2hi(e):
        # [128, (k,d)] bf16: w2[512+k*128+p, d]*2^15
        w = w2[e][512:].reshape(4, 128, D)           # [k, p, d]
        return np.ascontiguousarray(
            (w.transpose(1, 0, 2) * (SH * SW)).reshape(128, 4 * D)
        ).astype(io_dtype)

    def pack_w18(e):
        # [128, FB*2*128] e4m3: col (fb, i, j) = w1[i*128+p, fb*128+j]*SW
        w = w1[e][:256].reshape(2, 128, FB, 128)     # [i, p, fb, j]
        return np.ascontiguousarray(
            (w.transpose(1, 2, 0, 3) * SW).reshape(128, FB * 256)
        ).astype(ml_dtypes.float8_e4m3)

    def pack_w1ahi(e):
        # [128, FB*2*128] bf16: col (fb, kc, j) = w1[(kc+2)*128+p, ...]*2^15
        w = w1[e][256:].reshape(2, 128, FB, 128)
        return np.ascontiguousarray(
            (w.transpose(1, 2, 0, 3) * (SX * SW)).reshape(128, FB * 256)
        ).astype(io_dtype)

    in_maps = []
    for c in range(N_CORES):
        n0 = min(int(counts[c]), NT0)
        toks0 = slot_tok[cum[c]:cum[c] + n0][core_ord[c]]
        xt = np.zeros((D, NT), io_dtype)
        xt[:, :n0] = xf[toks0].astype(io_dtype).T
        gate = np.zeros(NT, np.float32)
        gate[:n0] = gates_sorted[cum[c]:cum[c] + n0][core_ord[c]]
        # fp8-mm2 chunk: its psum carries an extra 2^15 factor; fold the
        # compensation into the gate values for those blocks
        for cid in m2_ids:
            gate[offs[cid]:offs[cid + 1]] /= SH * SW
        if c in topup_by_core:
            te, lo, nb = topup_by_core[c]
            tt = slot_tok[cum[te] + lo:cum[te] + lo + nb]
            xt[:, NT0:NT0 + nb] = xf[tt].astype(io_dtype).T
            gate[NT0:NT0 + nb] = gates_sorted[cum[te] + lo:cum[te] + lo + nb]
            eb = te
        else:
            eb = 0  # unused slot1: gate=0 rows, any weights
        im = {
            "xp": permute_x(xt),
            "w1a": pack_w1(c), "w2a": pack_w2(c), "b1a": pack_b1(c),
            "w1b": pack_w1(eb), "w2b": pack_w2(eb), "b1b": pack_b1(eb),
            "gate2": np.ascontiguousarray(gate.reshape(NTG, 128).T),
        }
        if m_fp8:
            # x8 per fp8 chunk: [128, (i, t)] = x[d=i*128+p, tok]*SX, fp32
            # source (not the bf16 xt) to avoid double rounding
            x8parts = []
            for cid in fp8_ids:
                cs8 = chunks[cid]
                tk = toks0[offs[cid]:offs[cid] + cs8]
                xc = np.zeros((256, cs8), np.float32)
                xc[:, :len(tk)] = xf[tk].T[:256] * SX
                xr = xc.reshape(2, 128, cs8)                     # [i, p, t]
                x8parts.append(xr.transpose(1, 0, 2).reshape(128, 2 * cs8))
            im["xp8"] = np.ascontiguousarray(
                np.concatenate(x8parts, axis=1)).astype(ml_dtypes.float8_e4m3)
            im["w18"] = pack_w18(c)
            im["w1ahi"] = pack_w1ahi(c)
        if m2_ids:
            im["w28"] = pack_w28(c)
            im["w2hi"] = pack_w2hi(c)
            im["b1a8"] = np.ascontiguousarray(
                (b1[c] * SH).reshape(FB, 128).T.astype(np.float32))
        in_maps.append(im)

    def run_device():
        key = (NT0, NF)
        if key not in _PROGRAM_CACHE:
            _PROGRAM_CACHE[key] = _build_program(NT0, NF)
        nc = _PROGRAM_CACHE[key]
        res = run_bass_kernel_spmd(nc, in_maps, list(range(N_CORES)), trace=TRACE)
        if TRACE and res.exec_time_ns is not None:
            print(f"HW exec time: {res.exec_time_ns} ns")
        return [res.results[c]["y"] for c in range(N_CORES)]

    try:
        try:
            y_cores = run_device()
        except Exception:
            # transient device errors (e.g. NRT exec-unit unrecoverable)
            # are usually gone on retry with a freshly built program
            _PROGRAM_CACHE.clear()
            y_cores = run_device()
    except Exception as exc:
        # last resort: identical math on the host so the result is still
        # correct even if the accelerator path is down
        import sys
        print(f"device path failed twice ({exc!r}); computing FFN on host",
              file=sys.stderr)
        out_slots = np.zeros((T * TOP_K, D), np.float32)
        for e in range(E):
            n = int(counts[e])
            toks = slot_tok[cum[e]:cum[e] + n]
            h = np.maximum(xf[toks] @ w1[e] + b1[e], 0.0)
            y = (h @ w2[e]) * gates_sorted[cum[e]:cum[e] + n, None]
            out_slots[perm[cum[e]:cum[e] + n]] = y.astype(np.float32)
        out = out_slots.reshape(T, TOP_K, D).sum(axis=1)
        combine = np.zeros((T, E), np.float32)
        np.put_along_axis(combine, top_idx, gwts.astype(np.float32), axis=1)
        out += combine @ b2
        return out.reshape(B, S, D).astype(np.float32)

    # ---- unshard: scatter slots back, sum the K slots per token, add b2.
    out_slots = np.zeros((T * TOP_K, D), np.float32)
    for c in range(N_CORES):
        n0 = min(int(counts[c]), NT0)
        sl = np.arange(cum[c], cum[c] + n0)[core_ord[c]]
        out_slots[perm[sl]] = y_cores[c][:n0].astype(np.float32)
    for (c, e, lo, nb) in topups:
        out_slots[perm[cum[e] + lo:cum[e] + lo + nb]] = \
            y_cores[c][NT0:NT0 + nb].astype(np.float32)
    out = out_slots.reshape(T, TOP_K, D).sum(axis=1)

    # combine @ b2 (gate-weighted expert output biases)
    combine = np.zeros((T, E), np.float32)
    np.put_along_axis(combine, top_idx, gwts.astype(np.float32), axis=1)
    out += combine @ b2

    return out.reshape(B, S, D).astype(np.float32)



# revision 58
# speedup vs baseline: 1.0223x; 1.0081x over previous
"""MoE layer (E=8 experts, top-2 routing) on 8 Trainium2 NeuronCores.

Strategy: expert-parallel with a 2-slot load-balancing template. The host
computes the gating network in fp64 (logits = x @ wg + bg, top-2, softmax)
and dispatches token-slots to cores. Each core's SPMD program processes
  slot0: NT0 tokens with weight set A (the core's primary expert), then
  slot1: NT1=128 tokens with weight set B (a top-up block of whichever
         expert overflowed NT0 tokens -- host-assigned).
This pads every core to NT0+128 tokens instead of the global max expert
count rounded up (4224 vs 4480 for the reference input), cutting PE time.

Per core FFN:  y = relu(x_e @ w1[e] + b1[e]) @ w2[e], then rows scaled by
the gate weight on-device; the host scatter-adds the two slots per token
back together (plus the combine@b2 bias term).

Mixed precision: per core, slot0 is sorted by gate weight descending and
the last NF=1920 slots (the lowest-gate ones, all g<=0.5, as
384+512+512+256+256-token chunks) run mm1 with the d<256 contraction
half as fp8-e4m3 DoubleRow matmuls (2x PE throughput; quantization noise
there is damped by the small gate); the final 256-token chunk (lowest
gates of all) additionally runs mm2's f<512 half in fp8, with h written
directly in fp8 by the mm1 activation and the 2^15 psum scale folded
into host-pre-scaled gates. Measured rel-err 1.83e-2 against the 2e-2
gate (bf16-only is 3.9e-3; 384 mm2-fp8 tokens would reach ~1.98e-2,
NF=2048 mm1 would cross ~2.2e-2). The bf16 halves carry a 2^15 weight
pre-scale so one activation scale (2^-15) serves both; bf16 matmuls must
run FIRST in each psum accumulation group -- the reverse order returns
wrong psum contents on hardware.

Other hardware notes baked into the schedule:
  - PE DVFS: full clock arrives ~6us after first PE activity and decays
    on ~1us gaps, so dummy warmup matmuls (on a vector-memset tile) run
    from the engine-init floor (~8us) until the chunk0 DMA set lands
    (~13.3us). gpsimd engagement depresses the PE clock ~20% kernel-wide
    -- do not use it.
  - mm1 runs one chunk ahead of mm2 so the first mm2's w2a dependency
    has ~2 chunk-times of DMA slack.
  - The last 128-token block's mm2 is column-split into four quarters,
    alternating store rings, to shrink the post-last-matmul tail.

DMA plan (two HWDGE FIFO rings; each dma_start costs ~600ns of serial
descriptor-gen on its sequencer and SDMA execution begins ~8.2us in):
  scalar ring: x chunk0 lo-half, w1a fb0, b1a, gates, w2a(A)
  sync ring:   x0 hi-half, w1a fb1-7, x1, w2a(B), x2.., fp8 tiles,
               w1b, w2b, b1b, then output stores

All device inputs are host-permuted so every SBUF partition's data is one
contiguous DRAM run. Hardcoded problem shape: x [4,4096,512],
w1 [8,512,1024], w2 [8,1024,512], wg [512,8], top_k=2.
"""

import os
import numpy as np

B, S, D, F, E = 4, 4096, 512, 1024, 8
TOP_K = 2
N_CORES = 8
KD = D // 128   # contraction blocks for mm1
FB = F // 128   # F blocks (h partition blocks / mm2 contraction blocks)
NT1 = 128       # top-up slot tokens

TRACE = os.environ.get("MOE_TRACE", "0") == "1"

_PROGRAM_CACHE = {}


def _chunk_plan(NT0, nf=0):
    """Token chunk sizes: slot0 split into bf16 chunks (NT0-nf tokens:
    512s + one 128-multiple remainder) followed by fp8 chunks (nf tokens:
    one 128-multiple remainder + 512s), then the 128-token top-up chunk
    last (small tail). Returns (chunks, fp8_ids)."""
    bf = NT0 - nf
    chunks = [512] * (bf // 512)
    if bf % 512:
        chunks.append(bf % 512)
    nbf = len(chunks)
    if nf % 512:
        chunks.append(nf % 512)
    chunks += [512] * (nf // 512)
    # carve the last 256 fp8 tokens (the lowest gates of all) into their
    # own chunk whose mm2 also runs its f<512 contraction half in fp8
    m2_ids = []
    if nf >= 512 and chunks[-1] == 512:
        chunks[-1] = 256
        chunks.append(256)
        m2_ids = [len(chunks) - 1]
    fp8_ids = list(range(nbf, len(chunks)))
    chunks.append(NT1)
    return chunks, fp8_ids, m2_ids


def _build_program(NT0, nf):
    from concourse import bacc, tile, mybir

    dt = mybir.dt
    DT = dt.bfloat16

    nc = bacc.Bacc("TRN2", target_bir_lowering=False, debug=False)

    chunks, fp8_ids, m2_ids = _chunk_plan(NT0, nf)
    offs = [sum(chunks[:i]) for i in range(len(chunks) + 1)]
    NT = NT0 + NT1
    NTG = NT // 128
    n0 = len(chunks) - 1  # number of slot0 chunks
    m_fp8 = len(fp8_ids)

    # host-permuted inputs: per-partition contiguous runs
    # xp: per chunk c, [128, KD*cs] block at col KD*offs[c]
    xp_d = nc.dram_tensor("xp", [128, KD * NT], DT, kind="ExternalInput").ap()
    # w1a/w1b: fb-major: col = fb*(KD*128) + kc*128 + j
    w1a_d = nc.dram_tensor("w1a", [128, FB * KD * 128], DT, kind="ExternalInput").ap()
    w1b_d = nc.dram_tensor("w1b", [128, FB * KD * 128], DT, kind="ExternalInput").ap()
    # w2a/w2b: col = fb*D + d, partition p = f within fb block
    w2a_d = nc.dram_tensor("w2a", [128, FB * D], DT, kind="ExternalInput").ap()
    w2b_d = nc.dram_tensor("w2b", [128, FB * D], DT, kind="ExternalInput").ap()
    b1a_d = nc.dram_tensor("b1a", [128, FB], dt.float32, kind="ExternalInput").ap()
    b1b_d = nc.dram_tensor("b1b", [128, FB], dt.float32, kind="ExternalInput").ap()
    g_d = nc.dram_tensor("gate2", [128, NTG], dt.float32, kind="ExternalInput").ap()
    y_d = nc.dram_tensor("y", [NT, D], DT, kind="ExternalOutput").ap()
    if m_fp8:
        # fp8 chunk: x8[p, i*cs + t] = x[d=i*128+p, t]*32 (e4m3); chunks
        # packed back-to-back (2*cs cols each)
        xp8_d = nc.dram_tensor("xp8", [128, 2 * nf], dt.float8e4,
                               kind="ExternalInput").ap()
        # w18[p, fb*256 + i*128 + j] = w1[i*128+p, fb*128+j]*1024 (e4m3)
        w18_d = nc.dram_tensor("w18", [128, FB * 256], dt.float8e4,
                               kind="ExternalInput").ap()
        # w1ahi[p, fb*256 + kc*128 + j] = w1[(kc+2)*128+p, fb*128+j]*2^15
        w1ahi_d = nc.dram_tensor("w1ahi", [128, FB * 256], DT,
                                 kind="ExternalInput").ap()
    if m2_ids:
        # mm2-fp8 weights: w28[p, j*1024 + i*512 + d] = w2[(2j+i)*128+p, d]
        # * 1024 (e4m3, f<512); w2hi[p, k*512 + d] = w2[512+k*128+p, d]*2^15
        w28_d = nc.dram_tensor("w28", [128, 2048], dt.float8e4,
                               kind="ExternalInput").ap()
        w2hi_d = nc.dram_tensor("w2hi", [128, 4 * 512], DT,
                                kind="ExternalInput").ap()
        # b1 pre-scaled by SH=32 for the fp8-h activation
        b1a8_d = nc.dram_tensor("b1a8", [128, FB], dt.float32,
                                kind="ExternalInput").ap()

    with tile.TileContext(nc) as tc:
        with (
            tc.tile_pool(name="sb", bufs=1) as sbpool,
            tc.tile_pool(name="ps", bufs=4, space="PSUM") as pspool,
        ):
            wpool = xpool = sbpool
            ps1 = ps2 = pspool
            # ---- head DMA plan. SDMA execution only begins at ~8.2us and
            # early per-ring bandwidth is only ~64GB/s, so the first-chunk
            # critical mass (x0 + w1a fb blocks, ~1.5MB) is balanced across
            # both rings with completion granularity matching the chain
            # consumption order: scalar carries x0lo + w1afb0, sync carries
            # x0hi then per-fb w1a singles.
            cs0 = chunks[0]
            x0 = xpool.tile([128, KD * cs0], DT, tag="x0", name="x0")
            nc.scalar.dma_start(out=x0[0:64, :], in_=xp_d[0:64, 0:KD * cs0])
            nc.sync.dma_start(out=x0[64:128, :], in_=xp_d[64:128, 0:KD * cs0])
            w1a_fb = []
            t = wpool.tile([128, KD * 128], DT, tag="w1a_fb0", name="w1afb0")
            nc.scalar.dma_start(out=t[:], in_=w1a_d[:, 0:KD * 128])
            w1a_fb.append(t)
            for fb in range(1, FB):
                t = wpool.tile([128, KD * 128], DT, tag=f"w1a_fb{fb}",
                               name=f"w1afb{fb}")
                nc.sync.dma_start(
                    out=t[:], in_=w1a_d[:, fb * KD * 128:(fb + 1) * KD * 128])
                w1a_fb.append(t)

            def w1a_slice(fb, kc):
                return w1a_fb[fb][:, kc * 128:(kc + 1) * 128]

            # scalar ring continues: tiny b1a/g (needed by the first RELU),
            # then w2a for the first mm2
            b1a_sb = wpool.tile([128, FB], dt.float32)
            nc.scalar.dma_start(out=b1a_sb[:], in_=b1a_d[:])
            g_sb = wpool.tile([128, NTG], dt.float32)
            nc.scalar.dma_start(out=g_sb[:], in_=g_d[:])
            w2a_A = wpool.tile([128, (FB // 2) * D], DT, name="w2aA")
            nc.scalar.dma_start(out=w2a_A[:], in_=w2a_d[:, 0:(FB // 2) * D])

            # warmup: dummy matmuls on a vector-memset scratch tile keep the
            # PE busy from the engine-init floor (~8.2us) through the DVFS
            # ramp (full clock arrives ~6us after PE-busy-start) until the
            # first x/w tiles land (~11.4us); the scratch psum is never
            # read. vector memset: gpsimd engagement was measured to depress
            # the PE clock ~20% for the whole kernel, and scalar/sync must
            # not be delayed since they issue the DMA descriptor gens.
            # 2 big + 72 small dummies bridge the PE from the engine-init
            # floor (~8.0us) to the ~14.2us arrival of the chunk0 critical
            # mass: bigs at low clock ~1.1us, smalls at 107ns until the
            # clock maxes (~11us), 56ns after. Ending early costs a clock
            # drop (~2us re-ramp); ending late costs one small dummy.
            warm = wpool.tile([128, 512], DT)
            nc.vector.memset(warm[:], 0.0)
            for i in range(54):
                pw = ps2.tile([128, 512], dt.float32, tag="ps2", bufs=5)
                if i < 2:
                    nc.tensor.matmul(pw[:], warm[:, 0:128], warm[:],
                                     start=True, stop=True)
                else:
                    nc.tensor.matmul(pw[:, 0:128], warm[:, 0:128],
                                     warm[:, 0:128], start=True, stop=True)

            # ---- sync (SP) HWDGE ring: bulk loads continue, stores below.
            x_tiles = [x0]

            for c in range(1, len(chunks)):
                cs = chunks[c]
                if c == 1:
                    # two kc-half tiles in the same ring/FIFO slot so
                    # chunk1's mm1 can start on the first half (kc blocks
                    # are read in order)
                    xa = xpool.tile([128, 2 * cs], DT, tag="x1a", name="x1a")
                    nc.sync.dma_start(
                        out=xa[:], in_=xp_d[:, KD * offs[c]:KD * offs[c] + 2 * cs])
                    xb = xpool.tile([128, 2 * cs], DT, tag="x1b", name="x1b")
                    nc.sync.dma_start(
                        out=xb[:],
                        in_=xp_d[:, KD * offs[c] + 2 * cs:KD * (offs[c] + cs)])
                    x_tiles.append((xa, xb))
                elif c == 2:
                    # w2a_B rides between x1 and x2: needed by mm2(c0) which
                    # now runs after mm1(c1), so ~21us of slack
                    w2a_B = wpool.tile([128, (FB // 2) * D], DT, name="w2aB")
                    nc.sync.dma_start(out=w2a_B[:], in_=w2a_d[:, (FB // 2) * D:])
                    xt = xpool.tile([128, KD * cs], DT, tag=f"x{c}", name=f"x{c}")
                    nc.sync.dma_start(
                        out=xt[:], in_=xp_d[:, KD * offs[c]:KD * (offs[c] + cs)])
                    x_tiles.append(xt)
                elif c in fp8_ids:
                    # fp8 chunk: only the kc2-3 (d>=256) half comes from xp;
                    # the d<256 half arrives as fp8 via xp8 below
                    xt = xpool.tile([128, 2 * cs], DT, tag=f"x{c}", name=f"x{c}")
                    nc.sync.dma_start(
                        out=xt[:],
                        in_=xp_d[:, KD * offs[c] + 2 * cs:KD * (offs[c] + cs)])
                    x_tiles.append(xt)
                else:
                    xt = xpool.tile([128, KD * cs], DT, tag=f"x{c}", name=f"x{c}")
                    nc.sync.dma_start(
                        out=xt[:], in_=xp_d[:, KD * offs[c]:KD * (offs[c] + cs)])
                    x_tiles.append(xt)

            x8_tiles = {}
            if m_fp8:
                x8off = 0
                for c in fp8_ids:
                    cs8 = chunks[c]
                    t8 = xpool.tile([128, 2, cs8], dt.float8e4, name=f"x8c{c}")
                    nc.sync.dma_start(
                        out=t8[:], in_=xp8_d[:, x8off:x8off + 2 * cs8]
                        .rearrange("p (a b) -> p a b", a=2))
                    x8_tiles[c] = t8
                    x8off += 2 * cs8
                w18_sb = wpool.tile([128, FB, 2, 128], dt.float8e4, name="w18")
                nc.sync.dma_start(
                    out=w18_sb[:],
                    in_=w18_d[:].rearrange("p (f a b) -> p f a b", f=FB, a=2))
                w1ahi_sb = wpool.tile([128, FB, 2, 128], DT, name="w1ahi")
                nc.sync.dma_start(
                    out=w1ahi_sb[:],
                    in_=w1ahi_d[:].rearrange("p (f a b) -> p f a b", f=FB, a=2))
            if m2_ids:
                w28_sb = wpool.tile([128, 2, 2, 512], dt.float8e4, name="w28")
                nc.sync.dma_start(
                    out=w28_sb[:],
                    in_=w28_d[:].rearrange("p (j i d) -> p j i d", j=2, i=2))
                w2hi_sb = wpool.tile([128, 4, 512], DT, name="w2hi")
                nc.sync.dma_start(
                    out=w2hi_sb[:],
                    in_=w2hi_d[:].rearrange("p (k d) -> p k d", k=4))
                b1a8_sb = wpool.tile([128, FB], dt.float32)
                nc.sync.dma_start(out=b1a8_sb[:], in_=b1a8_d[:])

            if len(chunks) <= 2:  # tiny-NT0 fallback: w2a_B not yet emitted
                w2a_B = wpool.tile([128, (FB // 2) * D], DT, name="w2aB")
                nc.sync.dma_start(out=w2a_B[:], in_=w2a_d[:, (FB // 2) * D:])

            w1b_sb = wpool.tile([128, FB * KD * 128], DT)
            nc.sync.dma_start(out=w1b_sb[:], in_=w1b_d[:])
            w2b_sb = wpool.tile([128, FB * D], DT)
            nc.sync.dma_start(out=w2b_sb[:], in_=w2b_d[:])
            b1b_sb = wpool.tile([128, FB], dt.float32)
            nc.sync.dma_start(out=b1b_sb[:], in_=b1b_d[:])

            def w1_slice(c, fb, kc):
                if c < n0:
                    return w1a_slice(fb, kc)
                return w1b_sb[:, fb * KD * 128 + kc * 128:fb * KD * 128 + (kc + 1) * 128]

            def w2_slice(c, fb, c0=0, c1=D):
                if c >= n0:
                    return w2b_sb[:, fb * D + c0:fb * D + c1]
                t, f = (w2a_A, fb) if fb < FB // 2 else (w2a_B, fb - FB // 2)
                return t[:, f * D + c0:f * D + c1]

            h_tiles = {}

            def do_mm1(c):
                cs = chunks[c]
                x_sb = x_tiles[c]
                b1_sb = b1a_sb if c < n0 else b1b_sb
                h_sb = sbpool.tile([128, FB, cs], DT, tag="h", bufs=4)
                h_tiles[c] = h_sb
                for fb in range(FB):
                    p = ps1.tile([128, cs], dt.float32, tag="ps1", bufs=3)
                    for kc in range(KD):
                        if isinstance(x_sb, tuple) and len(x_sb) == KD:
                            xop = x_sb[kc][:]
                        elif isinstance(x_sb, tuple):
                            xt_, k_ = (x_sb[0], kc) if kc < 2 else (x_sb[1], kc - 2)
                            xop = xt_[:, k_ * cs:(k_ + 1) * cs]
                        else:
                            xop = x_sb[:, kc * cs:(kc + 1) * cs]
                        nc.tensor.matmul(
                            p[:],
                            w1_slice(c, fb, kc),
                            xop,
                            start=(kc == 0),
                            stop=(kc == KD - 1),
                        )
                    nc.scalar.activation(
                        h_sb[:, fb, :],
                        p[:],
                        mybir.ActivationFunctionType.Relu,
                        bias=b1_sb[:, fb:fb + 1],
                        scale=1.0,
                    )

            def do_mm1_fp8(c):
                # d<256 half of the contraction as one DoubleRow fp8 matmul
                # per (fb, token-half); d>=256 half in bf16 with weights
                # pre-scaled by 2^15 to match the fp8 product scale
                # (32*1024); the activation divides the sum back out.
                cs = chunks[c]
                x_hi = x_tiles[c]       # [128, 2*cs] bf16: kc2,kc3
                x_lo = x8_tiles[c]      # [128, 2, cs] fp8
                m2 = c in m2_ids
                if m2:
                    # h split by dtype: fb0-3 as fp8*32 (feeds mm2's DR
                    # half), fb4-7 bf16
                    h8_sb = sbpool.tile([128, 4, cs], dt.float8e4, tag="h8",
                                        bufs=2)
                    hb_sb = sbpool.tile([128, 4, cs], DT, tag="hb", bufs=2)
                    h_tiles[c] = (h8_sb, hb_sb)
                else:
                    h_sb = sbpool.tile([128, FB, cs], DT, tag="h", bufs=4)
                    h_tiles[c] = h_sb
                ths = [(t0, min(256, cs - t0)) for t0 in range(0, cs, 256)]
                for fb in range(FB):
                    p = ps1.tile([128, cs], dt.float32, tag="ps1", bufs=3)
                    # bf16 half FIRST, DR fp8 accumulating after: the
                    # reverse order (DR with start, bf16 accumulating)
                    # produces wrong psum contents on hardware
                    for kc in range(2):
                        nc.tensor.matmul(
                            p[:],
                            w1ahi_sb[:, fb, kc],
                            x_hi[:, kc * cs:(kc + 1) * cs],
                            start=(kc == 0), stop=False,
                            skip_group_check=True,
                        )
                    for ti, (t0, tw) in enumerate(ths):
                        nc.tensor.matmul(
                            p[:, t0:t0 + tw],
                            w18_sb[:, fb],
                            x_lo[:, :, t0:t0 + tw],
                            start=False, stop=(ti == len(ths) - 1),
                            perf_mode=mybir.MatmulPerfMode.DoubleRow,
                            skip_group_check=True,
                        )
                    if m2 and fb < 4:
                        # h8 = relu(pre*32): fold SH into the act scale
                        # (relu is positively homogeneous); bias = b1*32
                        nc.scalar.activation(
                            h8_sb[:, fb, :],
                            p[:],
                            mybir.ActivationFunctionType.Relu,
                            bias=b1a8_sb[:, fb:fb + 1],
                            scale=32.0 / 32768.0,
                        )
                    elif m2:
                        nc.scalar.activation(
                            hb_sb[:, fb - 4, :],
                            p[:],
                            mybir.ActivationFunctionType.Relu,
                            bias=b1a_sb[:, fb:fb + 1],
                            scale=1.0 / 32768.0,
                        )
                    else:
                        nc.scalar.activation(
                            h_sb[:, fb, :],
                            p[:],
                            mybir.ActivationFunctionType.Relu,
                            bias=b1a_sb[:, fb:fb + 1],
                            scale=1.0 / 32768.0,
                        )

            def do_mm2_fp8(c):
                # f<512 contraction half as DoubleRow fp8 (h8*32 x w2*1024),
                # f>=512 in bf16 with w2 pre-scaled 2^15; psum = 2^15 * y,
                # compensated by host-pre-scaled gates for these blocks.
                cs = chunks[c]
                h8_sb, hb_sb = h_tiles.pop(c)
                for tb in range(cs // 128):
                    blk = offs[c] // 128 + tb
                    r0 = offs[c] + tb * 128
                    p2 = ps2.tile([128, 512], dt.float32, tag="ps2", bufs=5)
                    for k in range(4):  # bf16 first (fb4-7)
                        nc.tensor.matmul(
                            p2[:],
                            hb_sb[:, k, tb * 128:(tb + 1) * 128],
                            w2hi_sb[:, k],
                            start=(k == 0), stop=False,
                            skip_group_check=True,
                        )
                    for j in range(2):
                        for q in range(2):
                            nc.tensor.matmul(
                                p2[:, q * 256:(q + 1) * 256],
                                h8_sb[:, 2 * j:2 * j + 2, tb * 128:(tb + 1) * 128],
                                w28_sb[:, j, :, q * 256:(q + 1) * 256],
                                start=False, stop=(j == 1 and q == 1),
                                perf_mode=mybir.MatmulPerfMode.DoubleRow,
                                skip_group_check=True,
                            )
                    o_sb = sbpool.tile([128, 512], DT, tag="o", bufs=16)
                    nc.vector.tensor_scalar_mul(
                        o_sb[:], p2[:], g_sb[:, blk:blk + 1]
                    )
                    nc.sync.dma_start(out=y_d[r0:r0 + 128, :], in_=o_sb[:])

            def do_mm2(c):
                cs = chunks[c]
                h_sb = h_tiles.pop(c)
                last_chunk = c == len(chunks) - 1
                for tb in range(cs // 128):
                    blk = offs[c] // 128 + tb
                    r0 = offs[c] + tb * 128
                    if not (last_chunk and tb == cs // 128 - 1):
                        p2 = ps2.tile([128, 512], dt.float32, tag="ps2", bufs=5)
                        for fb in range(FB):
                            nc.tensor.matmul(
                                p2[:],
                                h_sb[:, fb, tb * 128:(tb + 1) * 128],
                                w2_slice(c, fb),
                                start=(fb == 0),
                                stop=(fb == FB - 1),
                            )
                        o_sb = sbpool.tile([128, 512], DT, tag="o", bufs=16)
                        nc.vector.tensor_scalar_mul(
                            o_sb[:], p2[:], g_sb[:, blk:blk + 1]
                        )
                        nc.sync.dma_start(out=y_d[r0:r0 + 128, :], in_=o_sb[:])
                    else:
                        # final 128-token block: column-split mm2 into four
                        # quarters so the gate-scale + store of earlier
                        # quarters overlap mm2 of later ones, shrinking the
                        # post-last-matmul tail (store descriptor-gen is
                        # ~600ns serial per ring, so alternate rings)
                        for q, eng in ((0, nc.sync), (1, nc.scalar),
                                       (2, nc.sync), (3, nc.scalar)):
                            # reuse the regular ps2 slots ([128,512] tag) so
                            # PSUM stays within the 8-bank budget
                            p2 = ps2.tile([128, 512], dt.float32, tag="ps2", bufs=5)
                            for fb in range(FB):
                                nc.tensor.matmul(
                                    p2[:, 0:128],
                                    h_sb[:, fb, tb * 128:(tb + 1) * 128],
                                    w2_slice(c, fb, q * 128, (q + 1) * 128),
                                    start=(fb == 0),
                                    stop=(fb == FB - 1),
                                )
                            o_sb = sbpool.tile([128, 128], DT, tag="oh", bufs=4)
                            nc.vector.tensor_scalar_mul(
                                o_sb[:], p2[:, 0:128], g_sb[:, blk:blk + 1]
                            )
                            eng.dma_start(
                                out=y_d[r0:r0 + 128, q * 128:(q + 1) * 128],
                                in_=o_sb[:],
                            )

            # software pipeline: mm1 runs one chunk ahead of mm2, so the
            # first mm2's w2a dependency has ~2 chunk-times of DMA slack
            def do_mm2_any(c):
                if c in m2_ids:
                    do_mm2_fp8(c)
                else:
                    do_mm2(c)

            nchunks = len(chunks)
            for c in range(nchunks):
                if c in fp8_ids:
                    do_mm1_fp8(c)
                else:
                    do_mm1(c)
                if c >= 1:
                    do_mm2_any(c - 1)
            do_mm2_any(nchunks - 1)
    nc.compile()
    return nc


def _install_ntff_hook():
    """Register the axon NTFF profiling hook that run_bass_kernel_spmd
    (trace=True) looks for under antenv.axon_hooks; this container's antenv
    lacks that module, so recreate it via ctypes against libaxon_pjrt.so."""
    import sys, types, ctypes, contextlib

    if "antenv.axon_hooks" in sys.modules:
        return
    try:
        lib = ctypes.CDLL("/opt/axon/libaxon_pjrt.so")
    except OSError:
        return
    if not hasattr(lib, "axon_start_nrt_profile"):
        return
    lib.axon_start_nrt_profile.argtypes = [ctypes.POINTER(ctypes.c_int64), ctypes.c_size_t]
    lib.axon_start_nrt_profile.restype = ctypes.c_int64
    lib.axon_stop_nrt_profile.argtypes = [ctypes.c_char_p]
    lib.axon_stop_nrt_profile.restype = ctypes.c_int64

    @contextlib.contextmanager
    def _hook(output_dir, device_ids):
        import jax

        jax.devices()
        if device_ids:
            ids = (ctypes.c_int64 * len(device_ids))(*device_ids)
            rc = lib.axon_start_nrt_profile(ids, len(device_ids))
        else:
            rc = lib.axon_start_nrt_profile(None, 0)
        if rc != 0:
            raise RuntimeError(f"axon_start_nrt_profile rc={rc}")
        try:
            yield
        finally:
            n = lib.axon_stop_nrt_profile(str(output_dir).encode())
            print(f"profile: {n} ntff file(s) written to {output_dir}")

    mod = types.ModuleType("antenv.axon_hooks")
    _holder = {"h": _hook}
    mod.set_axon_ntff_profile_hook = lambda h: _holder.__setitem__("h", h)
    mod.get_axon_ntff_profile_hook = lambda: _holder["h"]
    sys.modules["antenv.axon_hooks"] = mod

    # avoid the S3/Fish artifact upload in the trace post-processing path
    import concourse.bass_utils as bu

    bu.upload_artifacts = lambda tmpdir: str(tmpdir)


def _pick_nt0(counts):
    """Smallest NT0 (multiple of 128) such that the overflow of every
    expert beyond NT0 fits in the 8 per-core 128-token top-up slots.
    Compare against the no-top-up template (pad all to max count)."""
    cmax = int(counts.max())
    nt_plain = max(512, -(-cmax // 128) * 128)
    best = None
    for nt0 in range(512, nt_plain + 128, 128):
        need = sum(-(-max(0, int(c) - nt0) // NT1) for c in counts)
        if need <= N_CORES:
            best = nt0
            break
    if best is None or best + NT1 >= nt_plain + NT1:
        best = nt_plain  # top-ups unused (gate=0 padding)
    return best


def kernel(**inputs):
    from concourse.bass_utils import run_bass_kernel_spmd
    import ml_dtypes

    if TRACE:
        _install_ntff_hook()

    x = np.asarray(inputs["x"], np.float32)
    w1 = np.asarray(inputs["w1"], np.float32)
    b1 = np.asarray(inputs["b1"], np.float32)
    w2 = np.asarray(inputs["w2"], np.float32)
    b2 = np.asarray(inputs["b2"], np.float32)
    wg = np.asarray(inputs["wg"], np.float32)
    bg = np.asarray(inputs["bg"], np.float32)

    T = x.shape[0] * x.shape[1]
    xf = x.reshape(T, D)

    # ---- host gating (fp64): logits -> top-2 (jax.lax.top_k tie order:
    # lower index wins -> stable argsort on -logits) -> softmax over top-2.
    logits = xf.astype(np.float64) @ wg.astype(np.float64) + bg.astype(np.float64)
    order = np.argsort(-logits, axis=1, kind="stable")
    top_idx = order[:, :TOP_K]                      # [T, K]
    top_vals = np.take_along_axis(logits, top_idx, axis=1)
    gwts = np.exp(top_vals - top_vals.max(axis=1, keepdims=True))
    gwts = gwts / gwts.sum(axis=1, keepdims=True)   # [T, K]

    # ---- dispatch: sort slots (t, k) by expert; per-expert contiguous runs.
    flat_expert = top_idx.ravel()                   # slot s = t*K + k
    perm = np.argsort(flat_expert, kind="stable")   # slots grouped by expert
    counts = np.bincount(flat_expert, minlength=E)
    cum = np.concatenate([[0], np.cumsum(counts)])
    slot_tok = perm // TOP_K                        # token of each sorted slot
    gates_sorted = gwts.ravel()[perm].astype(np.float32)

    NT0 = _pick_nt0(counts)
    NT = NT0 + NT1
    NTG = NT // 128

    # ---- mixed precision: per core, sort slot0 by gate descending; the
    # last NF slots (lowest gates, all g<=0.5) run mm1's d<256 half in fp8
    # DoubleRow -- quantization noise there is damped by the gate weight.
    # Measured rel-err 1.707e-2 at NF=1920 vs the 2e-2 gate (bf16 baseline
    # 3.9e-3; NF=2048+ would cross 2.2e-2).
    core_ord = []
    sec_min = NT
    for c in range(N_CORES):
        n0c = min(int(counts[c]), NT0)
        g = gates_sorted[cum[c]:cum[c] + n0c]
        og = np.argsort(-g, kind="stable")
        core_ord.append(og)
        sec_min = min(sec_min, int((g <= 0.5).sum()))
    NF = 128 * (min(1920, sec_min) // 128)
    chunks, fp8_ids, m2_ids = _chunk_plan(NT0, NF)
    offs = [sum(chunks[:i]) for i in range(len(chunks) + 1)]

    io_dtype = ml_dtypes.bfloat16
    w1_io = w1.astype(io_dtype)
    w2_io = w2.astype(io_dtype)

    # top-up assignment: expert e's slots beyond NT0, chopped into
    # 128-blocks, each block -> one core's slot1. record: (core, e, lo, n)
    topups = []
    next_core = 0
    for e in range(E):
        n = int(counts[e])
        for lo in range(NT0, n, NT1):
            nb = min(NT1, n - lo)
            assert next_core < N_CORES, "top-up slots exhausted"
            topups.append((next_core, e, lo, nb))
            next_core += 1
    topup_by_core = {c: (e, lo, nb) for (c, e, lo, nb) in topups}

    def permute_x(xt):
        # xt [D, NT] -> [128, KD*NT]: per chunk, (kc, token) contiguous
        xr = xt.reshape(KD, 128, NT)
        parts = [
            xr[:, :, offs[c]:offs[c + 1]].transpose(1, 0, 2).reshape(128, -1)
            for c in range(len(chunks))
        ]
        return np.ascontiguousarray(np.concatenate(parts, axis=1))

    def pack_w1(e):
        # [128, FB*KD*128] fb-major: col = fb*KD*128 + kc*128 + j
        w = w1_io[e].reshape(KD, 128, FB, 128)       # [kc, p, fb, j]
        return np.ascontiguousarray(
            w.transpose(1, 2, 0, 3).reshape(128, FB * KD * 128))

    def pack_w2(e):
        return np.ascontiguousarray(
            w2_io[e].reshape(FB, 128, D).transpose(1, 0, 2).reshape(128, FB * D))

    def pack_b1(e):
        return np.ascontiguousarray(b1[e].reshape(FB, 128).T)

    m_fp8 = len(fp8_ids)
    SX, SW = 32.0, 1024.0  # exact powers of two; bf16 half carries 2^15
    SH = 32.0              # h scale for the fp8-mm2 chunk

    def pack_w28(e):
        # [128, (j,i,d)] e4m3: w2[(2j+i)*128+p, d]*SW for f<512
        w = w2[e][:512].reshape(2, 2, 128, D)        # [j, i, p, d]
        return np.ascontiguousarray(
            (w.transpose(2, 0, 1, 3) * SW).reshape(128, 2048)
        ).astype(ml_dtypes.float8_e4m3)

    def pack_w2hi(e):
        # [128, (k,d)] bf16: w2[512+k*128+p, d]*2^15
        w = w2[e][512:].reshape(4, 128, D)           # [k, p, d]
        return np.ascontiguousarray(
            (w.transpose(1, 0, 2) * (SH * SW)).reshape(128, 4 * D)
        ).astype(io_dtype)

    def pack_w18(e):
        # [128, FB*2*128] e4m3: col (fb, i, j) = w1[i*128+p, fb*128+j]*SW
        w = w1[e][:256].reshape(2, 128, FB, 128)     # [i, p, fb, j]
        return np.ascontiguousarray(
            (w.transpose(1, 2, 0, 3) * SW).reshape(128, FB * 256)
        ).astype(ml_dtypes.float8_e4m3)

    def pack_w1ahi(e):
        # [128, FB*2*128] bf16: col (fb, kc, j) = w1[(kc+2)*128+p, ...]*2^15
        w = w1[e][256:].reshape(2, 128, FB, 128)
        return np.ascontiguousarray(
            (w.transpose(1, 2, 0, 3) * (SX * SW)).reshape(128, FB * 256)
        ).astype(io_dtype)

    in_maps = []
    for c in range(N_CORES):
        n0 = min(int(counts[c]), NT0)
        toks0 = slot_tok[cum[c]:cum[c] + n0][core_ord[c]]
        xt = np.zeros((D, NT), io_dtype)
        xt[:, :n0] = xf[toks0].astype(io_dtype).T
        gate = np.zeros(NT, np.float32)
        gate[:n0] = gates_sorted[cum[c]:cum[c] + n0][core_ord[c]]
        # fp8-mm2 chunk: its psum carries an extra 2^15 factor; fold the
        # compensation into the gate values for those blocks
        for cid in m2_ids:
            gate[offs[cid]:offs[cid + 1]] /= SH * SW
        if c in topup_by_core:
            te, lo, nb = topup_by_core[c]
            tt = slot_tok[cum[te] + lo:cum[te] + lo + nb]
            xt[:, NT0:NT0 + nb] = xf[tt].astype(io_dtype).T
            gate[NT0:NT0 + nb] = gates_sorted[cum[te] + lo:cum[te] + lo + nb]
            eb = te
        else:
            eb = 0  # unused slot1: gate=0 rows, any weights
        im = {
            "xp": permute_x(xt),
            "w1a": pack_w1(c), "w2a": pack_w2(c), "b1a": pack_b1(c),
            "w1b": pack_w1(eb), "w2b": pack_w2(eb), "b1b": pack_b1(eb),
            "gate2": np.ascontiguousarray(gate.reshape(NTG, 128).T),
        }
        if m_fp8:
            # x8 per fp8 chunk: [128, (i, t)] = x[d=i*128+p, tok]*SX, fp32
            # source (not the bf16 xt) to avoid double rounding
            x8parts = []
            for cid in fp8_ids:
                cs8 = chunks[cid]
                tk = toks0[offs[cid]:offs[cid] + cs8]
                xc = np.zeros((256, cs8), np.float32)
                xc[:, :len(tk)] = xf[tk].T[:256] * SX
                xr = xc.reshape(2, 128, cs8)                     # [i, p, t]
                x8parts.append(xr.transpose(1, 0, 2).reshape(128, 2 * cs8))
            im["xp8"] = np.ascontiguousarray(
                np.concatenate(x8parts, axis=1)).astype(ml_dtypes.float8_e4m3)
            im["w18"] = pack_w18(c)
            im["w1ahi"] = pack_w1ahi(c)
        if m2_ids:
            im["w28"] = pack_w28(c)
            im["w2hi"] = pack_w2hi(c)
            im["b1a8"] = np.ascontiguousarray(
                (b1[c] * SH).reshape(FB, 128).T.astype(np.float32))
        in_maps.append(im)

    def run_device():
        key = (NT0, NF)
        if key not in _PROGRAM_CACHE:
            _PROGRAM_CACHE[key] = _build_program(NT0, NF)
        nc = _PROGRAM_CACHE[key]
        res = run_bass_kernel_spmd(nc, in_maps, list(range(N_CORES)), trace=TRACE)
        if TRACE and res.exec_time_ns is not None:
            print(f"HW exec time: {res.exec_time_ns} ns")
        return [res.results[c]["y"] for c in range(N_CORES)]

    try:
        try:
            y_cores = run_device()
        except Exception:
            # transient device errors (e.g. NRT exec-unit unrecoverable)
            # are usually gone on retry with a freshly built program
            _PROGRAM_CACHE.clear()
            y_cores = run_device()
    except Exception as exc:
        # last resort: identical math on the host so the result is still
        # correct even if the accelerator path is down
        import sys
        print(f"device path failed twice ({exc!r}); computing FFN on host",
              file=sys.stderr)
        out_slots = np.zeros((T * TOP_K, D), np.float32)
        for e in range(E):
            n = int(counts[e])
            toks = slot_tok[cum[e]:cum[e] + n]
            h = np.maximum(xf[toks] @ w1[e] + b1[e], 0.0)
            y = (h @ w2[e]) * gates_sorted[cum[e]:cum[e] + n, None]
            out_slots[perm[cum[e]:cum[e] + n]] = y.astype(np.float32)
        out = out_slots.reshape(T, TOP_K, D).sum(axis=1)
        combine = np.zeros((T, E), np.float32)
        np.put_along_axis(combine, top_idx, gwts.astype(np.float32), axis=1)
        out += combine @ b2
        return out.reshape(B, S, D).astype(np.float32)

    # ---- unshard: scatter slots back, sum the K slots per token, add b2.
    out_slots = np.zeros((T * TOP_K, D), np.float32)
    for c in range(N_CORES):
        n0 = min(int(counts[c]), NT0)
        sl = np.arange(cum[c], cum[c] + n0)[core_ord[c]]
        out_slots[perm[sl]] = y_cores[c][:n0].astype(np.float32)
    for (c, e, lo, nb) in topups:
        out_slots[perm[cum[e] + lo:cum[e] + lo + nb]] = \
            y_cores[c][NT0:NT0 + nb].astype(np.float32)
    out = out_slots.reshape(T, TOP_K, D).sum(axis=1)

    # combine @ b2 (gate-weighted expert output biases)
    combine = np.zeros((T, E), np.float32)
    np.put_along_axis(combine, top_idx, gwts.astype(np.float32), axis=1)
    out += combine @ b2

    return out.reshape(B, S, D).astype(np.float32)



# revision 61
# speedup vs baseline: 1.0264x; 1.0040x over previous
"""MoE layer (E=8 experts, top-2 routing) on 8 Trainium2 NeuronCores.

Strategy: expert-parallel with a 2-slot load-balancing template. The host
computes the gating network in fp64 (logits = x @ wg + bg, top-2, softmax)
and dispatches token-slots to cores. Each core's SPMD program processes
  slot0: NT0 tokens with weight set A (the core's primary expert), then
  slot1: NT1=128 tokens with weight set B (a top-up block of whichever
         expert overflowed NT0 tokens -- host-assigned).
This pads every core to NT0+128 tokens instead of the global max expert
count rounded up (4224 vs 4480 for the reference input), cutting PE time.

Per core FFN:  y = relu(x_e @ w1[e] + b1[e]) @ w2[e], then rows scaled by
the gate weight on-device; the host scatter-adds the two slots per token
back together (plus the combine@b2 bias term).

Mixed precision: per core, slot0 is sorted by gate weight descending and
the last NF=1920 slots (the lowest-gate ones, all g<=0.5, as
384+512+512+256+256-token chunks) run mm1 with the d<256 contraction
half as fp8-e4m3 DoubleRow matmuls (2x PE throughput; quantization noise
there is damped by the small gate); the final 256-token chunk (lowest
gates of all) additionally runs mm2's f<512 half in fp8, with h written
directly in fp8 by the mm1 activation and the 2^15 psum scale folded
into host-pre-scaled gates. Measured rel-err 1.83e-2 against the 2e-2
gate (bf16-only is 3.9e-3; 384 mm2-fp8 tokens would reach ~1.98e-2,
NF=2048 mm1 would cross ~2.2e-2). The bf16 halves carry a 2^15 weight
pre-scale so one activation scale (2^-15) serves both; bf16 matmuls must
run FIRST in each psum accumulation group -- the reverse order returns
wrong psum contents on hardware.

Other hardware notes baked into the schedule:
  - PE DVFS: full clock arrives ~6us after first PE activity and decays
    on ~1us gaps, so dummy warmup matmuls (on a vector-memset tile) run
    from the engine-init floor (~8us) until the chunk0 DMA set lands
    (~13.3us). gpsimd engagement depresses the PE clock ~20% kernel-wide
    -- do not use it.
  - mm1 runs one chunk ahead of mm2 so the first mm2's w2a dependency
    has ~2 chunk-times of DMA slack.
  - The last 128-token block's mm2 is column-split into four quarters,
    alternating store rings, to shrink the post-last-matmul tail.

DMA plan (two HWDGE FIFO rings; each dma_start costs ~600ns of serial
descriptor-gen on its sequencer and SDMA execution begins ~8.2us in):
  scalar ring: x chunk0 lo-half, w1a fb0, b1a, gates, w2a(A)
  sync ring:   x0 hi-half, w1a fb1-7, x1, w2a(B), x2.., fp8 tiles,
               w1b, w2b, b1b, then output stores

All device inputs are host-permuted so every SBUF partition's data is one
contiguous DRAM run. Hardcoded problem shape: x [4,4096,512],
w1 [8,512,1024], w2 [8,1024,512], wg [512,8], top_k=2.
"""

import os
import numpy as np

B, S, D, F, E = 4, 4096, 512, 1024, 8
TOP_K = 2
N_CORES = 8
KD = D // 128   # contraction blocks for mm1
FB = F // 128   # F blocks (h partition blocks / mm2 contraction blocks)
NT1 = 128       # top-up slot tokens

TRACE = os.environ.get("MOE_TRACE", "0") == "1"

_PROGRAM_CACHE = {}


def _chunk_plan(NT0, nf=0):
    """Token chunk sizes: slot0 split into bf16 chunks (NT0-nf tokens:
    512s + one 128-multiple remainder) followed by fp8 chunks (nf tokens:
    one 128-multiple remainder + 512s), then the 128-token top-up chunk
    last (small tail). Returns (chunks, fp8_ids)."""
    bf = NT0 - nf
    chunks = [512] * (bf // 512)
    if bf % 512:
        chunks.append(bf % 512)
    nbf = len(chunks)
    if nf % 512:
        chunks.append(nf % 512)
    chunks += [512] * (nf // 512)
    # carve the last 256 fp8 tokens (the lowest gates of all) into their
    # own chunk whose mm2 also runs its f<512 contraction half in fp8
    m2_ids = []
    f8full_ids = []
    if nf >= 512 and chunks[-1] == 512:
        chunks[-1] = 256
        chunks.append(256)
        m2_ids = [len(chunks) - 1]
        f8full_ids = [len(chunks) - 2]
    fp8_ids = list(range(nbf, len(chunks)))
    chunks.append(NT1)
    return chunks, fp8_ids, m2_ids, f8full_ids


def _build_program(NT0, nf):
    from concourse import bacc, tile, mybir

    dt = mybir.dt
    DT = dt.bfloat16

    nc = bacc.Bacc("TRN2", target_bir_lowering=False, debug=False)

    chunks, fp8_ids, m2_ids, f8full_ids = _chunk_plan(NT0, nf)
    offs = [sum(chunks[:i]) for i in range(len(chunks) + 1)]
    NT = NT0 + NT1
    NTG = NT // 128
    n0 = len(chunks) - 1  # number of slot0 chunks
    m_fp8 = len(fp8_ids)

    # host-permuted inputs: per-partition contiguous runs
    # xp: per chunk c, [128, KD*cs] block at col KD*offs[c]
    xp_d = nc.dram_tensor("xp", [128, KD * NT], DT, kind="ExternalInput").ap()
    # w1a/w1b: fb-major: col = fb*(KD*128) + kc*128 + j
    w1a_d = nc.dram_tensor("w1a", [128, FB * KD * 128], DT, kind="ExternalInput").ap()
    w1b_d = nc.dram_tensor("w1b", [128, FB * KD * 128], DT, kind="ExternalInput").ap()
    # w2a/w2b: col = fb*D + d, partition p = f within fb block
    w2a_d = nc.dram_tensor("w2a", [128, FB * D], DT, kind="ExternalInput").ap()
    w2b_d = nc.dram_tensor("w2b", [128, FB * D], DT, kind="ExternalInput").ap()
    b1a_d = nc.dram_tensor("b1a", [128, FB], dt.float32, kind="ExternalInput").ap()
    b1b_d = nc.dram_tensor("b1b", [128, FB], dt.float32, kind="ExternalInput").ap()
    g_d = nc.dram_tensor("gate2", [128, NTG], dt.float32, kind="ExternalInput").ap()
    y_d = nc.dram_tensor("y", [NT, D], DT, kind="ExternalOutput").ap()
    if m_fp8:
        # fp8 chunk: x8[p, i*cs + t] = x[d=i*128+p, t]*32 (e4m3); chunks
        # packed back-to-back (2*cs cols each)
        xp8_d = nc.dram_tensor("xp8", [128, 2 * nf], dt.float8e4,
                               kind="ExternalInput").ap()
        # w18[p, fb*256 + i*128 + j] = w1[i*128+p, fb*128+j]*1024 (e4m3)
        w18_d = nc.dram_tensor("w18", [128, FB * 256], dt.float8e4,
                               kind="ExternalInput").ap()
        # w1ahi[p, fb*256 + kc*128 + j] = w1[(kc+2)*128+p, fb*128+j]*2^15
        w1ahi_d = nc.dram_tensor("w1ahi", [128, FB * 256], DT,
                                 kind="ExternalInput").ap()
    if m2_ids:
        # mm2-fp8 weights: w28[p, j*1024 + i*512 + d] = w2[(2j+i)*128+p, d]
        # * 1024 (e4m3, f<512); w2hi[p, k*512 + d] = w2[512+k*128+p, d]*2^15
        w28_d = nc.dram_tensor("w28", [128, 2048], dt.float8e4,
                               kind="ExternalInput").ap()
        w2hi_d = nc.dram_tensor("w2hi", [128, 4 * 512], DT,
                                kind="ExternalInput").ap()
        # b1 pre-scaled by SH=32 for the fp8-h activation
        b1a8_d = nc.dram_tensor("b1a8", [128, FB], dt.float32,
                                kind="ExternalInput").ap()
    if f8full_ids:
        # full-contraction fp8 mm1 for the c8 chunk: x all-d and w1 all-d
        xp8f_d = nc.dram_tensor("xp8f", [128, 4 * 256], dt.float8e4,
                                kind="ExternalInput").ap()
        w18f_d = nc.dram_tensor("w18f", [128, FB * 512], dt.float8e4,
                                kind="ExternalInput").ap()

    with tile.TileContext(nc) as tc:
        with (
            tc.tile_pool(name="sb", bufs=1) as sbpool,
            tc.tile_pool(name="ps", bufs=4, space="PSUM") as pspool,
        ):
            wpool = xpool = sbpool
            ps1 = ps2 = pspool
            # ---- head DMA plan. SDMA execution only begins at ~8.2us and
            # early per-ring bandwidth is only ~64GB/s, so the first-chunk
            # critical mass (x0 + w1a fb blocks, ~1.5MB) is balanced across
            # both rings with completion granularity matching the chain
            # consumption order: scalar carries x0lo + w1afb0, sync carries
            # x0hi then per-fb w1a singles.
            cs0 = chunks[0]
            x0 = xpool.tile([128, KD * cs0], DT, tag="x0", name="x0")
            nc.scalar.dma_start(out=x0[0:64, :], in_=xp_d[0:64, 0:KD * cs0])
            nc.sync.dma_start(out=x0[64:128, :], in_=xp_d[64:128, 0:KD * cs0])
            w1a_fb = []
            t = wpool.tile([128, KD * 128], DT, tag="w1a_fb0", name="w1afb0")
            nc.scalar.dma_start(out=t[:], in_=w1a_d[:, 0:KD * 128])
            w1a_fb.append(t)
            for fb in range(1, FB):
                t = wpool.tile([128, KD * 128], DT, tag=f"w1a_fb{fb}",
                               name=f"w1afb{fb}")
                nc.sync.dma_start(
                    out=t[:], in_=w1a_d[:, fb * KD * 128:(fb + 1) * KD * 128])
                w1a_fb.append(t)

            def w1a_slice(fb, kc):
                return w1a_fb[fb][:, kc * 128:(kc + 1) * 128]

            # scalar ring continues: tiny b1a/g (needed by the first RELU),
            # then w2a for the first mm2
            b1a_sb = wpool.tile([128, FB], dt.float32)
            nc.scalar.dma_start(out=b1a_sb[:], in_=b1a_d[:])
            g_sb = wpool.tile([128, NTG], dt.float32)
            nc.scalar.dma_start(out=g_sb[:], in_=g_d[:])
            w2a_A = wpool.tile([128, (FB // 2) * D], DT, name="w2aA")
            nc.scalar.dma_start(out=w2a_A[:], in_=w2a_d[:, 0:(FB // 2) * D])

            # warmup: dummy matmuls on a vector-memset scratch tile keep the
            # PE busy from the engine-init floor (~8.2us) through the DVFS
            # ramp (full clock arrives ~6us after PE-busy-start) until the
            # first x/w tiles land (~11.4us); the scratch psum is never
            # read. vector memset: gpsimd engagement was measured to depress
            # the PE clock ~20% for the whole kernel, and scalar/sync must
            # not be delayed since they issue the DMA descriptor gens.
            # 2 big + 72 small dummies bridge the PE from the engine-init
            # floor (~8.0us) to the ~14.2us arrival of the chunk0 critical
            # mass: bigs at low clock ~1.1us, smalls at 107ns until the
            # clock maxes (~11us), 56ns after. Ending early costs a clock
            # drop (~2us re-ramp); ending late costs one small dummy.
            warm = wpool.tile([128, 512], DT)
            nc.vector.memset(warm[:], 0.0)
            for i in range(54):
                pw = ps2.tile([128, 512], dt.float32, tag="ps2", bufs=5)
                if i < 2:
                    nc.tensor.matmul(pw[:], warm[:, 0:128], warm[:],
                                     start=True, stop=True)
                else:
                    nc.tensor.matmul(pw[:, 0:128], warm[:, 0:128],
                                     warm[:, 0:128], start=True, stop=True)

            # ---- sync (SP) HWDGE ring: bulk loads continue, stores below.
            x_tiles = [x0]

            for c in range(1, len(chunks)):
                cs = chunks[c]
                if c == 1:
                    # two kc-half tiles in the same ring/FIFO slot so
                    # chunk1's mm1 can start on the first half (kc blocks
                    # are read in order)
                    xa = xpool.tile([128, 2 * cs], DT, tag="x1a", name="x1a")
                    nc.sync.dma_start(
                        out=xa[:], in_=xp_d[:, KD * offs[c]:KD * offs[c] + 2 * cs])
                    xb = xpool.tile([128, 2 * cs], DT, tag="x1b", name="x1b")
                    nc.sync.dma_start(
                        out=xb[:],
                        in_=xp_d[:, KD * offs[c] + 2 * cs:KD * (offs[c] + cs)])
                    x_tiles.append((xa, xb))
                elif c == 2:
                    # w2a_B rides between x1 and x2: needed by mm2(c0) which
                    # now runs after mm1(c1), so ~21us of slack
                    w2a_B = wpool.tile([128, (FB // 2) * D], DT, name="w2aB")
                    nc.sync.dma_start(out=w2a_B[:], in_=w2a_d[:, (FB // 2) * D:])
                    xt = xpool.tile([128, KD * cs], DT, tag=f"x{c}", name=f"x{c}")
                    nc.sync.dma_start(
                        out=xt[:], in_=xp_d[:, KD * offs[c]:KD * (offs[c] + cs)])
                    x_tiles.append(xt)
                elif c in fp8_ids:
                    # fp8 chunk: only the kc2-3 (d>=256) half comes from xp;
                    # the d<256 half arrives as fp8 via xp8 below. The
                    # full-fp8 chunk needs no bf16 x at all.
                    if c in f8full_ids:
                        x_tiles.append(None)
                        continue
                    xt = xpool.tile([128, 2 * cs], DT, tag=f"x{c}", name=f"x{c}")
                    nc.sync.dma_start(
                        out=xt[:],
                        in_=xp_d[:, KD * offs[c] + 2 * cs:KD * (offs[c] + cs)])
                    x_tiles.append(xt)
                else:
                    xt = xpool.tile([128, KD * cs], DT, tag=f"x{c}", name=f"x{c}")
                    nc.sync.dma_start(
                        out=xt[:], in_=xp_d[:, KD * offs[c]:KD * (offs[c] + cs)])
                    x_tiles.append(xt)

            x8_tiles = {}
            if m_fp8:
                x8off = 0
                for c in fp8_ids:
                    cs8 = chunks[c]
                    if c in f8full_ids:
                        x8off += 2 * cs8   # region unused; comes via xp8f
                        continue
                    t8 = xpool.tile([128, 2, cs8], dt.float8e4, name=f"x8c{c}")
                    nc.sync.dma_start(
                        out=t8[:], in_=xp8_d[:, x8off:x8off + 2 * cs8]
                        .rearrange("p (a b) -> p a b", a=2))
                    x8_tiles[c] = t8
                    x8off += 2 * cs8
                w18_sb = wpool.tile([128, FB, 2, 128], dt.float8e4, name="w18")
                nc.sync.dma_start(
                    out=w18_sb[:],
                    in_=w18_d[:].rearrange("p (f a b) -> p f a b", f=FB, a=2))
                w1ahi_sb = wpool.tile([128, FB, 2, 128], DT, name="w1ahi")
                nc.sync.dma_start(
                    out=w1ahi_sb[:],
                    in_=w1ahi_d[:].rearrange("p (f a b) -> p f a b", f=FB, a=2))
            if m2_ids:
                w28_sb = wpool.tile([128, 2, 2, 512], dt.float8e4, name="w28")
                nc.sync.dma_start(
                    out=w28_sb[:],
                    in_=w28_d[:].rearrange("p (j i d) -> p j i d", j=2, i=2))
                w2hi_sb = wpool.tile([128, 4, 512], DT, name="w2hi")
                nc.sync.dma_start(
                    out=w2hi_sb[:],
                    in_=w2hi_d[:].rearrange("p (k d) -> p k d", k=4))
                b1a8_sb = wpool.tile([128, FB], dt.float32)
                nc.sync.dma_start(out=b1a8_sb[:], in_=b1a8_d[:])
            if f8full_ids:
                x8f_sb = xpool.tile([128, 4, 256], dt.float8e4, name="x8f")
                nc.sync.dma_start(
                    out=x8f_sb[:],
                    in_=xp8f_d[:].rearrange("p (a b) -> p a b", a=4))
                w18f_sb = wpool.tile([128, FB, 4, 128], dt.float8e4,
                                     name="w18f")
                nc.sync.dma_start(
                    out=w18f_sb[:],
                    in_=w18f_d[:].rearrange("p (f a b) -> p f a b", f=FB, a=4))

            if len(chunks) <= 2:  # tiny-NT0 fallback: w2a_B not yet emitted
                w2a_B = wpool.tile([128, (FB // 2) * D], DT, name="w2aB")
                nc.sync.dma_start(out=w2a_B[:], in_=w2a_d[:, (FB // 2) * D:])

            w1b_sb = wpool.tile([128, FB * KD * 128], DT)
            nc.sync.dma_start(out=w1b_sb[:], in_=w1b_d[:])
            w2b_sb = wpool.tile([128, FB * D], DT)
            nc.sync.dma_start(out=w2b_sb[:], in_=w2b_d[:])
            b1b_sb = wpool.tile([128, FB], dt.float32)
            nc.sync.dma_start(out=b1b_sb[:], in_=b1b_d[:])

            def w1_slice(c, fb, kc):
                if c < n0:
                    return w1a_slice(fb, kc)
                return w1b_sb[:, fb * KD * 128 + kc * 128:fb * KD * 128 + (kc + 1) * 128]

            def w2_slice(c, fb, c0=0, c1=D):
                if c >= n0:
                    return w2b_sb[:, fb * D + c0:fb * D + c1]
                t, f = (w2a_A, fb) if fb < FB // 2 else (w2a_B, fb - FB // 2)
                return t[:, f * D + c0:f * D + c1]

            h_tiles = {}

            def do_mm1(c):
                cs = chunks[c]
                x_sb = x_tiles[c]
                b1_sb = b1a_sb if c < n0 else b1b_sb
                h_sb = sbpool.tile([128, FB, cs], DT, tag="h", bufs=4)
                h_tiles[c] = h_sb
                for fb in range(FB):
                    p = ps1.tile([128, cs], dt.float32, tag="ps1", bufs=3)
                    for kc in range(KD):
                        if isinstance(x_sb, tuple) and len(x_sb) == KD:
                            xop = x_sb[kc][:]
                        elif isinstance(x_sb, tuple):
                            xt_, k_ = (x_sb[0], kc) if kc < 2 else (x_sb[1], kc - 2)
                            xop = xt_[:, k_ * cs:(k_ + 1) * cs]
                        else:
                            xop = x_sb[:, kc * cs:(kc + 1) * cs]
                        nc.tensor.matmul(
                            p[:],
                            w1_slice(c, fb, kc),
                            xop,
                            start=(kc == 0),
                            stop=(kc == KD - 1),
                        )
                    nc.scalar.activation(
                        h_sb[:, fb, :],
                        p[:],
                        mybir.ActivationFunctionType.Relu,
                        bias=b1_sb[:, fb:fb + 1],
                        scale=1.0,
                    )

            def do_mm1_fp8(c):
                # d<256 half of the contraction as one DoubleRow fp8 matmul
                # per (fb, token-half); d>=256 half in bf16 with weights
                # pre-scaled by 2^15 to match the fp8 product scale
                # (32*1024); the activation divides the sum back out.
                cs = chunks[c]
                full = c in f8full_ids
                x_hi = x_tiles[c]       # [128, 2*cs] bf16: kc2,kc3
                x_lo = None if full else x8_tiles[c]  # [128, 2, cs] fp8
                m2 = c in m2_ids
                if m2:
                    # h split by dtype: fb0-3 as fp8*32 (feeds mm2's DR
                    # half), fb4-7 bf16
                    h8_sb = sbpool.tile([128, 4, cs], dt.float8e4, tag="h8",
                                        bufs=2)
                    hb_sb = sbpool.tile([128, 4, cs], DT, tag="hb", bufs=2)
                    h_tiles[c] = (h8_sb, hb_sb)
                else:
                    h_sb = sbpool.tile([128, FB, cs], DT, tag="h", bufs=4)
                    h_tiles[c] = h_sb
                ths = [(t0, min(256, cs - t0)) for t0 in range(0, cs, 256)]
                for fb in range(FB):
                    p = ps1.tile([128, cs], dt.float32, tag="ps1", bufs=3)
                    if full:
                        # whole contraction as two chained DoubleRow fp8
                        for hf in range(2):
                            nc.tensor.matmul(
                                p[:],
                                w18f_sb[:, fb, 2 * hf:2 * hf + 2],
                                x8f_sb[:, 2 * hf:2 * hf + 2, :],
                                start=(hf == 0), stop=(hf == 1),
                                perf_mode=mybir.MatmulPerfMode.DoubleRow,
                                skip_group_check=True,
                            )
                        nc.scalar.activation(
                            h_sb[:, fb, :], p[:],
                            mybir.ActivationFunctionType.Relu,
                            bias=b1a_sb[:, fb:fb + 1], scale=1.0 / 32768.0)
                        continue
                    # bf16 half FIRST, DR fp8 accumulating after: the
                    # reverse order (DR with start, bf16 accumulating)
                    # produces wrong psum contents on hardware
                    for kc in range(2):
                        nc.tensor.matmul(
                            p[:],
                            w1ahi_sb[:, fb, kc],
                            x_hi[:, kc * cs:(kc + 1) * cs],
                            start=(kc == 0), stop=False,
                            skip_group_check=True,
                        )
                    for ti, (t0, tw) in enumerate(ths):
                        nc.tensor.matmul(
                            p[:, t0:t0 + tw],
                            w18_sb[:, fb],
                            x_lo[:, :, t0:t0 + tw],
                            start=False, stop=(ti == len(ths) - 1),
                            perf_mode=mybir.MatmulPerfMode.DoubleRow,
                            skip_group_check=True,
                        )
                    if m2 and fb < 4:
                        # h8 = relu(pre*32): fold SH into the act scale
                        # (relu is positively homogeneous); bias = b1*32
                        nc.scalar.activation(
                            h8_sb[:, fb, :],
                            p[:],
                            mybir.ActivationFunctionType.Relu,
                            bias=b1a8_sb[:, fb:fb + 1],
                            scale=32.0 / 32768.0,
                        )
                    elif m2:
                        nc.scalar.activation(
                            hb_sb[:, fb - 4, :],
                            p[:],
                            mybir.ActivationFunctionType.Relu,
                            bias=b1a_sb[:, fb:fb + 1],
                            scale=1.0 / 32768.0,
                        )
                    else:
                        nc.scalar.activation(
                            h_sb[:, fb, :],
                            p[:],
                            mybir.ActivationFunctionType.Relu,
                            bias=b1a_sb[:, fb:fb + 1],
                            scale=1.0 / 32768.0,
                        )

            def do_mm2_fp8(c):
                # f<512 contraction half as DoubleRow fp8 (h8*32 x w2*1024),
                # f>=512 in bf16 with w2 pre-scaled 2^15; psum = 2^15 * y,
                # compensated by host-pre-scaled gates for these blocks.
                cs = chunks[c]
                h8_sb, hb_sb = h_tiles.pop(c)
                for tb in range(cs // 128):
                    blk = offs[c] // 128 + tb
                    r0 = offs[c] + tb * 128
                    p2 = ps2.tile([128, 512], dt.float32, tag="ps2", bufs=5)
                    for k in range(4):  # bf16 first (fb4-7)
                        nc.tensor.matmul(
                            p2[:],
                            hb_sb[:, k, tb * 128:(tb + 1) * 128],
                            w2hi_sb[:, k],
                            start=(k == 0), stop=False,
                            skip_group_check=True,
                        )
                    for j in range(2):
                        for q in range(2):
                            nc.tensor.matmul(
                                p2[:, q * 256:(q + 1) * 256],
                                h8_sb[:, 2 * j:2 * j + 2, tb * 128:(tb + 1) * 128],
                                w28_sb[:, j, :, q * 256:(q + 1) * 256],
                                start=False, stop=(j == 1 and q == 1),
                                perf_mode=mybir.MatmulPerfMode.DoubleRow,
                                skip_group_check=True,
                            )
                    o_sb = sbpool.tile([128, 512], DT, tag="o", bufs=16)
                    nc.vector.tensor_scalar_mul(
                        o_sb[:], p2[:], g_sb[:, blk:blk + 1]
                    )
                    nc.sync.dma_start(out=y_d[r0:r0 + 128, :], in_=o_sb[:])

            def do_mm2(c):
                cs = chunks[c]
                h_sb = h_tiles.pop(c)
                last_chunk = c == len(chunks) - 1
                for tb in range(cs // 128):
                    blk = offs[c] // 128 + tb
                    r0 = offs[c] + tb * 128
                    if not (last_chunk and tb == cs // 128 - 1):
                        p2 = ps2.tile([128, 512], dt.float32, tag="ps2", bufs=5)
                        for fb in range(FB):
                            nc.tensor.matmul(
                                p2[:],
                                h_sb[:, fb, tb * 128:(tb + 1) * 128],
                                w2_slice(c, fb),
                                start=(fb == 0),
                                stop=(fb == FB - 1),
                            )
                        o_sb = sbpool.tile([128, 512], DT, tag="o", bufs=16)
                        nc.vector.tensor_scalar_mul(
                            o_sb[:], p2[:], g_sb[:, blk:blk + 1]
                        )
                        nc.sync.dma_start(out=y_d[r0:r0 + 128, :], in_=o_sb[:])
                    else:
                        # final 128-token block: column-split mm2 into four
                        # quarters so the gate-scale + store of earlier
                        # quarters overlap mm2 of later ones, shrinking the
                        # post-last-matmul tail (store descriptor-gen is
                        # ~600ns serial per ring, so alternate rings)
                        for q, eng in ((0, nc.sync), (1, nc.scalar),
                                       (2, nc.sync), (3, nc.scalar)):
                            # reuse the regular ps2 slots ([128,512] tag) so
                            # PSUM stays within the 8-bank budget
                            p2 = ps2.tile([128, 512], dt.float32, tag="ps2", bufs=5)
                            for fb in range(FB):
                                nc.tensor.matmul(
                                    p2[:, 0:128],
                                    h_sb[:, fb, tb * 128:(tb + 1) * 128],
                                    w2_slice(c, fb, q * 128, (q + 1) * 128),
                                    start=(fb == 0),
                                    stop=(fb == FB - 1),
                                )
                            o_sb = sbpool.tile([128, 128], DT, tag="oh", bufs=4)
                            nc.vector.tensor_scalar_mul(
                                o_sb[:], p2[:, 0:128], g_sb[:, blk:blk + 1]
                            )
                            eng.dma_start(
                                out=y_d[r0:r0 + 128, q * 128:(q + 1) * 128],
                                in_=o_sb[:],
                            )

            # software pipeline: mm1 runs one chunk ahead of mm2, so the
            # first mm2's w2a dependency has ~2 chunk-times of DMA slack
            def do_mm2_any(c):
                if c in m2_ids:
                    do_mm2_fp8(c)
                else:
                    do_mm2(c)

            nchunks = len(chunks)
            for c in range(nchunks):
                if c in fp8_ids:
                    do_mm1_fp8(c)
                else:
                    do_mm1(c)
                if c >= 1:
                    do_mm2_any(c - 1)
            do_mm2_any(nchunks - 1)
    nc.compile()
    return nc


def _install_ntff_hook():
    """Register the axon NTFF profiling hook that run_bass_kernel_spmd
    (trace=True) looks for under antenv.axon_hooks; this container's antenv
    lacks that module, so recreate it via ctypes against libaxon_pjrt.so."""
    import sys, types, ctypes, contextlib

    if "antenv.axon_hooks" in sys.modules:
        return
    try:
        lib = ctypes.CDLL("/opt/axon/libaxon_pjrt.so")
    except OSError:
        return
    if not hasattr(lib, "axon_start_nrt_profile"):
        return
    lib.axon_start_nrt_profile.argtypes = [ctypes.POINTER(ctypes.c_int64), ctypes.c_size_t]
    lib.axon_start_nrt_profile.restype = ctypes.c_int64
    lib.axon_stop_nrt_profile.argtypes = [ctypes.c_char_p]
    lib.axon_stop_nrt_profile.restype = ctypes.c_int64

    @contextlib.contextmanager
    def _hook(output_dir, device_ids):
        import jax

        jax.devices()
        if device_ids:
            ids = (ctypes.c_int64 * len(device_ids))(*device_ids)
            rc = lib.axon_start_nrt_profile(ids, len(device_ids))
        else:
            rc = lib.axon_start_nrt_profile(None, 0)
        if rc != 0:
            raise RuntimeError(f"axon_start_nrt_profile rc={rc}")
        try:
            yield
        finally:
            n = lib.axon_stop_nrt_profile(str(output_dir).encode())
            print(f"profile: {n} ntff file(s) written to {output_dir}")

    mod = types.ModuleType("antenv.axon_hooks")
    _holder = {"h": _hook}
    mod.set_axon_ntff_profile_hook = lambda h: _holder.__setitem__("h", h)
    mod.get_axon_ntff_profile_hook = lambda: _holder["h"]
    sys.modules["antenv.axon_hooks"] = mod

    # avoid the S3/Fish artifact upload in the trace post-processing path
    import concourse.bass_utils as bu

    bu.upload_artifacts = lambda tmpdir: str(tmpdir)


def _pick_nt0(counts):
    """Smallest NT0 (multiple of 128) such that the overflow of every
    expert beyond NT0 fits in the 8 per-core 128-token top-up slots.
    Compare against the no-top-up template (pad all to max count)."""
    cmax = int(counts.max())
    nt_plain = max(512, -(-cmax // 128) * 128)
    best = None
    for nt0 in range(512, nt_plain + 128, 128):
        need = sum(-(-max(0, int(c) - nt0) // NT1) for c in counts)
        if need <= N_CORES:
            best = nt0
            break
    if best is None or best + NT1 >= nt_plain + NT1:
        best = nt_plain  # top-ups unused (gate=0 padding)
    return best


def kernel(**inputs):
    from concourse.bass_utils import run_bass_kernel_spmd
    import ml_dtypes

    if TRACE:
        _install_ntff_hook()

    x = np.asarray(inputs["x"], np.float32)
    w1 = np.asarray(inputs["w1"], np.float32)
    b1 = np.asarray(inputs["b1"], np.float32)
    w2 = np.asarray(inputs["w2"], np.float32)
    b2 = np.asarray(inputs["b2"], np.float32)
    wg = np.asarray(inputs["wg"], np.float32)
    bg = np.asarray(inputs["bg"], np.float32)

    T = x.shape[0] * x.shape[1]
    xf = x.reshape(T, D)

    # ---- host gating (fp64): logits -> top-2 (jax.lax.top_k tie order:
    # lower index wins -> stable argsort on -logits) -> softmax over top-2.
    logits = xf.astype(np.float64) @ wg.astype(np.float64) + bg.astype(np.float64)
    order = np.argsort(-logits, axis=1, kind="stable")
    top_idx = order[:, :TOP_K]                      # [T, K]
    top_vals = np.take_along_axis(logits, top_idx, axis=1)
    gwts = np.exp(top_vals - top_vals.max(axis=1, keepdims=True))
    gwts = gwts / gwts.sum(axis=1, keepdims=True)   # [T, K]

    # ---- dispatch: sort slots (t, k) by expert; per-expert contiguous runs.
    flat_expert = top_idx.ravel()                   # slot s = t*K + k
    perm = np.argsort(flat_expert, kind="stable")   # slots grouped by expert
    counts = np.bincount(flat_expert, minlength=E)
    cum = np.concatenate([[0], np.cumsum(counts)])
    slot_tok = perm // TOP_K                        # token of each sorted slot
    gates_sorted = gwts.ravel()[perm].astype(np.float32)

    NT0 = _pick_nt0(counts)
    NT = NT0 + NT1
    NTG = NT // 128

    # ---- mixed precision: per core, sort slot0 by gate descending; the
    # last NF slots (lowest gates, all g<=0.5) run mm1's d<256 half in fp8
    # DoubleRow -- quantization noise there is damped by the gate weight.
    # Measured rel-err 1.707e-2 at NF=1920 vs the 2e-2 gate (bf16 baseline
    # 3.9e-3; NF=2048+ would cross 2.2e-2).
    core_ord = []
    sec_min = NT
    for c in range(N_CORES):
        n0c = min(int(counts[c]), NT0)
        g = gates_sorted[cum[c]:cum[c] + n0c]
        og = np.argsort(-g, kind="stable")
        core_ord.append(og)
        sec_min = min(sec_min, int((g <= 0.5).sum()))
    NF = 128 * (min(1920, sec_min) // 128)
    chunks, fp8_ids, m2_ids, f8full_ids = _chunk_plan(NT0, NF)
    offs = [sum(chunks[:i]) for i in range(len(chunks) + 1)]

    io_dtype = ml_dtypes.bfloat16
    w1_io = w1.astype(io_dtype)
    w2_io = w2.astype(io_dtype)

    # top-up assignment: expert e's slots beyond NT0, chopped into
    # 128-blocks, each block -> one core's slot1. record: (core, e, lo, n)
    topups = []
    next_core = 0
    for e in range(E):
        n = int(counts[e])
        for lo in range(NT0, n, NT1):
            nb = min(NT1, n - lo)
            assert next_core < N_CORES, "top-up slots exhausted"
            topups.append((next_core, e, lo, nb))
            next_core += 1
    topup_by_core = {c: (e, lo, nb) for (c, e, lo, nb) in topups}

    def permute_x(xt):
        # xt [D, NT] -> [128, KD*NT]: per chunk, (kc, token) contiguous
        xr = xt.reshape(KD, 128, NT)
        parts = [
            xr[:, :, offs[c]:offs[c + 1]].transpose(1, 0, 2).reshape(128, -1)
            for c in range(len(chunks))
        ]
        return np.ascontiguousarray(np.concatenate(parts, axis=1))

    def pack_w1(e):
        # [128, FB*KD*128] fb-major: col = fb*KD*128 + kc*128 + j
        w = w1_io[e].reshape(KD, 128, FB, 128)       # [kc, p, fb, j]
        return np.ascontiguousarray(
            w.transpose(1, 2, 0, 3).reshape(128, FB * KD * 128))

    def pack_w2(e):
        return np.ascontiguousarray(
            w2_io[e].reshape(FB, 128, D).transpose(1, 0, 2).reshape(128, FB * D))

    def pack_b1(e):
        return np.ascontiguousarray(b1[e].reshape(FB, 128).T)

    m_fp8 = len(fp8_ids)
    SX, SW = 32.0, 1024.0  # exact powers of two; bf16 half carries 2^15
    SH = 32.0              # h scale for the fp8-mm2 chunk

    def pack_w28(e):
        # [128, (j,i,d)] e4m3: w2[(2j+i)*128+p, d]*SW for f<512
        w = w2[e][:512].reshape(2, 2, 128, D)        # [j, i, p, d]
        return np.ascontiguousarray(
            (w.transpose(2, 0, 1, 3) * SW).reshape(128, 2048)
        ).astype(ml_dtypes.float8_e4m3)

    def pack_w2hi(e):
        # [128, (k,d)] bf16: w2[512+k*128+p, d]*2^15
        w = w2[e][512:].reshape(4, 128, D)           # [k, p, d]
        return np.ascontiguousarray(
            (w.transpose(1, 0, 2) * (SH * SW)).reshape(128, 4 * D)
        ).astype(io_dtype)

    def pack_w18(e):
        # [128, FB*2*128] e4m3: col (fb, i, j) = w1[i*128+p, fb*128+j]*SW
        w = w1[e][:256].reshape(2, 128, FB, 128)     # [i, p, fb, j]
        return np.ascontiguousarray(
            (w.transpose(1, 2, 0, 3) * SW).reshape(128, FB * 256)
        ).astype(ml_dtypes.float8_e4m3)

    def pack_w1ahi(e):
        # [128, FB*2*128] bf16: col (fb, kc, j) = w1[(kc+2)*128+p, ...]*2^15
        w = w1[e][256:].reshape(2, 128, FB, 128)
        return np.ascontiguousarray(
            (w.transpose(1, 2, 0, 3) * (SX * SW)).reshape(128, FB * 256)
        ).astype(io_dtype)

    in_maps = []
    for c in range(N_CORES):
        n0 = min(int(counts[c]), NT0)
        toks0 = slot_tok[cum[c]:cum[c] + n0][core_ord[c]]
        xt = np.zeros((D, NT), io_dtype)
        xt[:, :n0] = xf[toks0].astype(io_dtype).T
        gate = np.zeros(NT, np.float32)
        gate[:n0] = gates_sorted[cum[c]:cum[c] + n0][core_ord[c]]
        # fp8-mm2 chunk: its psum carries an extra 2^15 factor; fold the
        # compensation into the gate values for those blocks
        for cid in m2_ids:
            gate[offs[cid]:offs[cid + 1]] /= SH * SW
        if c in topup_by_core:
            te, lo, nb = topup_by_core[c]
            tt = slot_tok[cum[te] + lo:cum[te] + lo + nb]
            xt[:, NT0:NT0 + nb] = xf[tt].astype(io_dtype).T
            gate[NT0:NT0 + nb] = gates_sorted[cum[te] + lo:cum[te] + lo + nb]
            eb = te
        else:
            eb = 0  # unused slot1: gate=0 rows, any weights
        im = {
            "xp": permute_x(xt),
            "w1a": pack_w1(c), "w2a": pack_w2(c), "b1a": pack_b1(c),
            "w1b": pack_w1(eb), "w2b": pack_w2(eb), "b1b": pack_b1(eb),
            "gate2": np.ascontiguousarray(gate.reshape(NTG, 128).T),
        }
        if m_fp8:
            # x8 per fp8 chunk: [128, (i, t)] = x[d=i*128+p, tok]*SX, fp32
            # source (not the bf16 xt) to avoid double rounding
            x8parts = []
            for cid in fp8_ids:
                cs8 = chunks[cid]
                tk = toks0[offs[cid]:offs[cid] + cs8]
                xc = np.zeros((256, cs8), np.float32)
                xc[:, :len(tk)] = xf[tk].T[:256] * SX
                xr = xc.reshape(2, 128, cs8)                     # [i, p, t]
                x8parts.append(xr.transpose(1, 0, 2).reshape(128, 2 * cs8))
            im["xp8"] = np.ascontiguousarray(
                np.concatenate(x8parts, axis=1)).astype(ml_dtypes.float8_e4m3)
            im["w18"] = pack_w18(c)
            im["w1ahi"] = pack_w1ahi(c)
        if m2_ids:
            im["w28"] = pack_w28(c)
            im["w2hi"] = pack_w2hi(c)
            im["b1a8"] = np.ascontiguousarray(
                (b1[c] * SH).reshape(FB, 128).T.astype(np.float32))
        if f8full_ids:
            cid = f8full_ids[0]
            tk = toks0[offs[cid]:offs[cid] + 256]
            xc = np.zeros((512, 256), np.float32)
            xc[:, :len(tk)] = xf[tk].T * SX
            im["xp8f"] = np.ascontiguousarray(
                xc.reshape(4, 128, 256).transpose(1, 0, 2).reshape(128, 1024)
            ).astype(ml_dtypes.float8_e4m3)
            wf = w1[c].reshape(4, 128, FB, 128)      # [i, p, fb, j]
            im["w18f"] = np.ascontiguousarray(
                (wf.transpose(1, 2, 0, 3) * SW).reshape(128, FB * 512)
            ).astype(ml_dtypes.float8_e4m3)
        in_maps.append(im)

    def run_device():
        key = (NT0, NF)
        if key not in _PROGRAM_CACHE:
            _PROGRAM_CACHE[key] = _build_program(NT0, NF)
        nc = _PROGRAM_CACHE[key]
        res = run_bass_kernel_spmd(nc, in_maps, list(range(N_CORES)), trace=TRACE)
        if TRACE and res.exec_time_ns is not None:
            print(f"HW exec time: {res.exec_time_ns} ns")
        return [res.results[c]["y"] for c in range(N_CORES)]

    try:
        try:
            y_cores = run_device()
        except Exception:
            # transient device errors (e.g. NRT exec-unit unrecoverable)
            # are usually gone on retry with a freshly built program
            _PROGRAM_CACHE.clear()
            y_cores = run_device()
    except Exception as exc:
        # last resort: identical math on the host so the result is still
        # correct even if the accelerator path is down
        import sys
        print(f"device path failed twice ({exc!r}); computing FFN on host",
              file=sys.stderr)
        out_slots = np.zeros((T * TOP_K, D), np.float32)
        for e in range(E):
            n = int(counts[e])
            toks = slot_tok[cum[e]:cum[e] + n]
            h = np.maximum(xf[toks] @ w1[e] + b1[e], 0.0)
            y = (h @ w2[e]) * gates_sorted[cum[e]:cum[e] + n, None]
            out_slots[perm[cum[e]:cum[e] + n]] = y.astype(np.float32)
        out = out_slots.reshape(T, TOP_K, D).sum(axis=1)
        combine = np.zeros((T, E), np.float32)
        np.put_along_axis(combine, top_idx, gwts.astype(np.float32), axis=1)
        out += combine @ b2
        return out.reshape(B, S, D).astype(np.float32)

    # ---- unshard: scatter slots back, sum the K slots per token, add b2.
    out_slots = np.zeros((T * TOP_K, D), np.float32)
    for c in range(N_CORES):
        n0 = min(int(counts[c]), NT0)
        sl = np.arange(cum[c], cum[c] + n0)[core_ord[c]]
        out_slots[perm[sl]] = y_cores[c][:n0].astype(np.float32)
    for (c, e, lo, nb) in topups:
        out_slots[perm[cum[e] + lo:cum[e] + lo + nb]] = \
            y_cores[c][NT0:NT0 + nb].astype(np.float32)
    out = out_slots.reshape(T, TOP_K, D).sum(axis=1)

    # combine @ b2 (gate-weighted expert output biases)
    combine = np.zeros((T, E), np.float32)
    np.put_along_axis(combine, top_idx, gwts.astype(np.float32), axis=1)
    out += combine @ b2

    return out.reshape(B, S, D).astype(np.float32)



# revision 62
# speedup vs baseline: 1.0271x; 1.0007x over previous
"""MoE layer (E=8 experts, top-2 routing) on 8 Trainium2 NeuronCores.

Strategy: expert-parallel with a 2-slot load-balancing template. The host
computes the gating network in fp64 (logits = x @ wg + bg, top-2, softmax)
and dispatches token-slots to cores. Each core's SPMD program processes
  slot0: NT0 tokens with weight set A (the core's primary expert), then
  slot1: NT1=128 tokens with weight set B (a top-up block of whichever
         expert overflowed NT0 tokens -- host-assigned).
This pads every core to NT0+128 tokens instead of the global max expert
count rounded up (4224 vs 4480 for the reference input), cutting PE time.

Per core FFN:  y = relu(x_e @ w1[e] + b1[e]) @ w2[e], then rows scaled by
the gate weight on-device; the host scatter-adds the two slots per token
back together (plus the combine@b2 bias term).

Mixed precision: per core, slot0 is sorted by gate weight descending and
the last NF=1920 slots (the lowest-gate ones, all g<=0.5, as
384+512+512+256+256-token chunks) run mm1 with the d<256 contraction
half as fp8-e4m3 DoubleRow matmuls (2x PE throughput; quantization noise
there is damped by the small gate); the final 256-token chunk (lowest
gates of all) additionally runs mm2's f<512 half in fp8, with h written
directly in fp8 by the mm1 activation and the 2^15 psum scale folded
into host-pre-scaled gates. Measured rel-err 1.83e-2 against the 2e-2
gate (bf16-only is 3.9e-3; 384 mm2-fp8 tokens would reach ~1.98e-2,
NF=2048 mm1 would cross ~2.2e-2). The bf16 halves carry a 2^15 weight
pre-scale so one activation scale (2^-15) serves both; bf16 matmuls must
run FIRST in each psum accumulation group -- the reverse order returns
wrong psum contents on hardware.

Other hardware notes baked into the schedule:
  - PE DVFS: full clock arrives ~6us after first PE activity and decays
    on ~1us gaps, so dummy warmup matmuls (on a vector-memset tile) run
    from the engine-init floor (~8us) until the chunk0 DMA set lands
    (~13.3us). gpsimd engagement depresses the PE clock ~20% kernel-wide
    -- do not use it.
  - mm1 runs one chunk ahead of mm2 so the first mm2's w2a dependency
    has ~2 chunk-times of DMA slack.
  - The last 128-token block's mm2 is column-split into four quarters,
    alternating store rings, to shrink the post-last-matmul tail.

DMA plan (two HWDGE FIFO rings; each dma_start costs ~600ns of serial
descriptor-gen on its sequencer and SDMA execution begins ~8.2us in):
  scalar ring: x chunk0 lo-half, w1a fb0, b1a, gates, w2a(A)
  sync ring:   x0 hi-half, w1a fb1-7, x1, w2a(B), x2.., fp8 tiles,
               w1b, w2b, b1b, then output stores

All device inputs are host-permuted so every SBUF partition's data is one
contiguous DRAM run. Hardcoded problem shape: x [4,4096,512],
w1 [8,512,1024], w2 [8,1024,512], wg [512,8], top_k=2.
"""

import os
import numpy as np

B, S, D, F, E = 4, 4096, 512, 1024, 8
TOP_K = 2
N_CORES = 8
KD = D // 128   # contraction blocks for mm1
FB = F // 128   # F blocks (h partition blocks / mm2 contraction blocks)
NT1 = 128       # top-up slot tokens

TRACE = os.environ.get("MOE_TRACE", "0") == "1"

_PROGRAM_CACHE = {}


def _chunk_plan(NT0, nf=0):
    """Token chunk sizes: slot0 split into bf16 chunks (NT0-nf tokens:
    512s + one 128-multiple remainder) followed by fp8 chunks (nf tokens:
    one 128-multiple remainder + 512s), then the 128-token top-up chunk
    last (small tail). Returns (chunks, fp8_ids)."""
    bf = NT0 - nf
    chunks = [512] * (bf // 512)
    if bf % 512:
        chunks.append(bf % 512)
    nbf = len(chunks)
    if nf % 512:
        chunks.append(nf % 512)
    chunks += [512] * (nf // 512)
    # carve the last 256 fp8 tokens (the lowest gates of all) into their
    # own chunk whose mm2 also runs its f<512 contraction half in fp8
    m2_ids = []
    f8full_ids = []
    if nf >= 512 and chunks[-1] == 512:
        chunks[-1] = 256
        chunks.append(256)
        m2_ids = [len(chunks) - 1]
        f8full_ids = [len(chunks) - 2]
    fp8_ids = list(range(nbf, len(chunks)))
    chunks.append(NT1)
    return chunks, fp8_ids, m2_ids, f8full_ids


def _build_program(NT0, nf):
    from concourse import bacc, tile, mybir

    dt = mybir.dt
    DT = dt.bfloat16

    nc = bacc.Bacc("TRN2", target_bir_lowering=False, debug=False)

    chunks, fp8_ids, m2_ids, f8full_ids = _chunk_plan(NT0, nf)
    offs = [sum(chunks[:i]) for i in range(len(chunks) + 1)]
    NT = NT0 + NT1
    NTG = NT // 128
    n0 = len(chunks) - 1  # number of slot0 chunks
    m_fp8 = len(fp8_ids)

    # host-permuted inputs: per-partition contiguous runs
    # xp: per chunk c, [128, KD*cs] block at col KD*offs[c]
    xp_d = nc.dram_tensor("xp", [128, KD * NT], DT, kind="ExternalInput").ap()
    # w1a/w1b: fb-major: col = fb*(KD*128) + kc*128 + j
    w1a_d = nc.dram_tensor("w1a", [128, FB * KD * 128], DT, kind="ExternalInput").ap()
    w1b_d = nc.dram_tensor("w1b", [128, FB * KD * 128], DT, kind="ExternalInput").ap()
    # w2a/w2b: col = fb*D + d, partition p = f within fb block
    w2a_d = nc.dram_tensor("w2a", [128, FB * D], DT, kind="ExternalInput").ap()
    w2b_d = nc.dram_tensor("w2b", [128, FB * D], DT, kind="ExternalInput").ap()
    b1a_d = nc.dram_tensor("b1a", [128, FB], dt.float32, kind="ExternalInput").ap()
    b1b_d = nc.dram_tensor("b1b", [128, FB], dt.float32, kind="ExternalInput").ap()
    g_d = nc.dram_tensor("gate2", [128, NTG], dt.float32, kind="ExternalInput").ap()
    y_d = nc.dram_tensor("y", [NT, D], DT, kind="ExternalOutput").ap()
    if m_fp8:
        # fp8 chunk: x8[p, i*cs + t] = x[d=i*128+p, t]*32 (e4m3); chunks
        # packed back-to-back (2*cs cols each)
        xp8_d = nc.dram_tensor("xp8", [128, 2 * nf], dt.float8e4,
                               kind="ExternalInput").ap()
        # w18[p, fb*256 + i*128 + j] = w1[i*128+p, fb*128+j]*1024 (e4m3)
        w18_d = nc.dram_tensor("w18", [128, FB * 256], dt.float8e4,
                               kind="ExternalInput").ap()
        # w1ahi[p, fb*256 + kc*128 + j] = w1[(kc+2)*128+p, fb*128+j]*2^15
        w1ahi_d = nc.dram_tensor("w1ahi", [128, FB * 256], DT,
                                 kind="ExternalInput").ap()
    if m2_ids:
        # mm2-fp8 weights: w28[p, j*1024 + i*512 + d] = w2[(2j+i)*128+p, d]
        # * 1024 (e4m3, f<512); w2hi[p, k*512 + d] = w2[512+k*128+p, d]*2^15
        w28_d = nc.dram_tensor("w28", [128, 2048], dt.float8e4,
                               kind="ExternalInput").ap()
        w2hi_d = nc.dram_tensor("w2hi", [128, 4 * 512], DT,
                                kind="ExternalInput").ap()
        # b1 pre-scaled by SH=32 for the fp8-h activation
        b1a8_d = nc.dram_tensor("b1a8", [128, FB], dt.float32,
                                kind="ExternalInput").ap()
    if f8full_ids:
        # full-contraction fp8 mm1 for the c8 chunk: x all-d and w1 all-d
        xp8f_d = nc.dram_tensor("xp8f", [128, 4 * 256], dt.float8e4,
                                kind="ExternalInput").ap()
        w18f_d = nc.dram_tensor("w18f", [128, FB * 512], dt.float8e4,
                                kind="ExternalInput").ap()

    with tile.TileContext(nc) as tc:
        with (
            tc.tile_pool(name="sb", bufs=1) as sbpool,
            tc.tile_pool(name="ps", bufs=4, space="PSUM") as pspool,
        ):
            wpool = xpool = sbpool
            ps1 = ps2 = pspool
            # ---- head DMA plan. SDMA execution only begins at ~8.2us and
            # early per-ring bandwidth is only ~64GB/s, so the first-chunk
            # critical mass (x0 + w1a fb blocks, ~1.5MB) is balanced across
            # both rings with completion granularity matching the chain
            # consumption order: scalar carries x0lo + w1afb0, sync carries
            # x0hi then per-fb w1a singles.
            cs0 = chunks[0]
            x0 = xpool.tile([128, KD * cs0], DT, tag="x0", name="x0")
            nc.scalar.dma_start(out=x0[0:64, :], in_=xp_d[0:64, 0:KD * cs0])
            nc.sync.dma_start(out=x0[64:128, :], in_=xp_d[64:128, 0:KD * cs0])
            w1a_fb = []
            t = wpool.tile([128, KD * 128], DT, tag="w1a_fb0", name="w1afb0")
            nc.scalar.dma_start(out=t[:], in_=w1a_d[:, 0:KD * 128])
            w1a_fb.append(t)
            for fb in range(1, FB):
                t = wpool.tile([128, KD * 128], DT, tag=f"w1a_fb{fb}",
                               name=f"w1afb{fb}")
                nc.sync.dma_start(
                    out=t[:], in_=w1a_d[:, fb * KD * 128:(fb + 1) * KD * 128])
                w1a_fb.append(t)

            def w1a_slice(fb, kc):
                return w1a_fb[fb][:, kc * 128:(kc + 1) * 128]

            # scalar ring continues: tiny b1a/g (needed by the first RELU),
            # then w2a for the first mm2
            b1a_sb = wpool.tile([128, FB], dt.float32)
            nc.scalar.dma_start(out=b1a_sb[:], in_=b1a_d[:])
            g_sb = wpool.tile([128, NTG], dt.float32)
            nc.scalar.dma_start(out=g_sb[:], in_=g_d[:])
            w2a_A = wpool.tile([128, (FB // 2) * D], DT, name="w2aA")
            nc.scalar.dma_start(out=w2a_A[:], in_=w2a_d[:, 0:(FB // 2) * D])

            # warmup: dummy matmuls on a vector-memset scratch tile keep the
            # PE busy from the engine-init floor (~8.2us) through the DVFS
            # ramp (full clock arrives ~6us after PE-busy-start) until the
            # first x/w tiles land (~11.4us); the scratch psum is never
            # read. vector memset: gpsimd engagement was measured to depress
            # the PE clock ~20% for the whole kernel, and scalar/sync must
            # not be delayed since they issue the DMA descriptor gens.
            # 2 big + 72 small dummies bridge the PE from the engine-init
            # floor (~8.0us) to the ~14.2us arrival of the chunk0 critical
            # mass: bigs at low clock ~1.1us, smalls at 107ns until the
            # clock maxes (~11us), 56ns after. Ending early costs a clock
            # drop (~2us re-ramp); ending late costs one small dummy.
            warm = wpool.tile([128, 512], DT)
            nc.vector.memset(warm[:], 0.0)
            for i in range(58):
                pw = ps2.tile([128, 512], dt.float32, tag="ps2", bufs=5)
                if i < 2:
                    nc.tensor.matmul(pw[:], warm[:, 0:128], warm[:],
                                     start=True, stop=True)
                else:
                    nc.tensor.matmul(pw[:, 0:128], warm[:, 0:128],
                                     warm[:, 0:128], start=True, stop=True)

            # ---- sync (SP) HWDGE ring: bulk loads continue, stores below.
            x_tiles = [x0]

            for c in range(1, len(chunks)):
                cs = chunks[c]
                if c == 1:
                    # two kc-half tiles in the same ring/FIFO slot so
                    # chunk1's mm1 can start on the first half (kc blocks
                    # are read in order)
                    xa = xpool.tile([128, 2 * cs], DT, tag="x1a", name="x1a")
                    nc.sync.dma_start(
                        out=xa[:], in_=xp_d[:, KD * offs[c]:KD * offs[c] + 2 * cs])
                    xb = xpool.tile([128, 2 * cs], DT, tag="x1b", name="x1b")
                    nc.sync.dma_start(
                        out=xb[:],
                        in_=xp_d[:, KD * offs[c] + 2 * cs:KD * (offs[c] + cs)])
                    x_tiles.append((xa, xb))
                elif c == 2:
                    # w2a_B rides between x1 and x2: needed by mm2(c0) which
                    # now runs after mm1(c1), so ~21us of slack
                    w2a_B = wpool.tile([128, (FB // 2) * D], DT, name="w2aB")
                    nc.sync.dma_start(out=w2a_B[:], in_=w2a_d[:, (FB // 2) * D:])
                    xt = xpool.tile([128, KD * cs], DT, tag=f"x{c}", name=f"x{c}")
                    nc.sync.dma_start(
                        out=xt[:], in_=xp_d[:, KD * offs[c]:KD * (offs[c] + cs)])
                    x_tiles.append(xt)
                elif c in fp8_ids:
                    # fp8 chunk: only the kc2-3 (d>=256) half comes from xp;
                    # the d<256 half arrives as fp8 via xp8 below. The
                    # full-fp8 chunk needs no bf16 x at all.
                    if c in f8full_ids:
                        x_tiles.append(None)
                        continue
                    xt = xpool.tile([128, 2 * cs], DT, tag=f"x{c}", name=f"x{c}")
                    nc.sync.dma_start(
                        out=xt[:],
                        in_=xp_d[:, KD * offs[c] + 2 * cs:KD * (offs[c] + cs)])
                    x_tiles.append(xt)
                else:
                    xt = xpool.tile([128, KD * cs], DT, tag=f"x{c}", name=f"x{c}")
                    nc.sync.dma_start(
                        out=xt[:], in_=xp_d[:, KD * offs[c]:KD * (offs[c] + cs)])
                    x_tiles.append(xt)

            x8_tiles = {}
            if m_fp8:
                x8off = 0
                for c in fp8_ids:
                    cs8 = chunks[c]
                    if c in f8full_ids:
                        x8off += 2 * cs8   # region unused; comes via xp8f
                        continue
                    t8 = xpool.tile([128, 2, cs8], dt.float8e4, name=f"x8c{c}")
                    nc.sync.dma_start(
                        out=t8[:], in_=xp8_d[:, x8off:x8off + 2 * cs8]
                        .rearrange("p (a b) -> p a b", a=2))
                    x8_tiles[c] = t8
                    x8off += 2 * cs8
                w18_sb = wpool.tile([128, FB, 2, 128], dt.float8e4, name="w18")
                nc.sync.dma_start(
                    out=w18_sb[:],
                    in_=w18_d[:].rearrange("p (f a b) -> p f a b", f=FB, a=2))
                w1ahi_sb = wpool.tile([128, FB, 2, 128], DT, name="w1ahi")
                nc.sync.dma_start(
                    out=w1ahi_sb[:],
                    in_=w1ahi_d[:].rearrange("p (f a b) -> p f a b", f=FB, a=2))
            if m2_ids:
                w28_sb = wpool.tile([128, 2, 2, 512], dt.float8e4, name="w28")
                nc.sync.dma_start(
                    out=w28_sb[:],
                    in_=w28_d[:].rearrange("p (j i d) -> p j i d", j=2, i=2))
                w2hi_sb = wpool.tile([128, 4, 512], DT, name="w2hi")
                nc.sync.dma_start(
                    out=w2hi_sb[:],
                    in_=w2hi_d[:].rearrange("p (k d) -> p k d", k=4))
                b1a8_sb = wpool.tile([128, FB], dt.float32)
                nc.sync.dma_start(out=b1a8_sb[:], in_=b1a8_d[:])
            if f8full_ids:
                x8f_sb = xpool.tile([128, 4, 256], dt.float8e4, name="x8f")
                nc.sync.dma_start(
                    out=x8f_sb[:],
                    in_=xp8f_d[:].rearrange("p (a b) -> p a b", a=4))
                w18f_sb = wpool.tile([128, FB, 4, 128], dt.float8e4,
                                     name="w18f")
                nc.sync.dma_start(
                    out=w18f_sb[:],
                    in_=w18f_d[:].rearrange("p (f a b) -> p f a b", f=FB, a=4))

            if len(chunks) <= 2:  # tiny-NT0 fallback: w2a_B not yet emitted
                w2a_B = wpool.tile([128, (FB // 2) * D], DT, name="w2aB")
                nc.sync.dma_start(out=w2a_B[:], in_=w2a_d[:, (FB // 2) * D:])

            w1b_sb = wpool.tile([128, FB * KD * 128], DT)
            nc.sync.dma_start(out=w1b_sb[:], in_=w1b_d[:])
            w2b_sb = wpool.tile([128, FB * D], DT)
            nc.sync.dma_start(out=w2b_sb[:], in_=w2b_d[:])
            b1b_sb = wpool.tile([128, FB], dt.float32)
            nc.sync.dma_start(out=b1b_sb[:], in_=b1b_d[:])

            def w1_slice(c, fb, kc):
                if c < n0:
                    return w1a_slice(fb, kc)
                return w1b_sb[:, fb * KD * 128 + kc * 128:fb * KD * 128 + (kc + 1) * 128]

            def w2_slice(c, fb, c0=0, c1=D):
                if c >= n0:
                    return w2b_sb[:, fb * D + c0:fb * D + c1]
                t, f = (w2a_A, fb) if fb < FB // 2 else (w2a_B, fb - FB // 2)
                return t[:, f * D + c0:f * D + c1]

            h_tiles = {}

            def do_mm1(c):
                cs = chunks[c]
                x_sb = x_tiles[c]
                b1_sb = b1a_sb if c < n0 else b1b_sb
                h_sb = sbpool.tile([128, FB, cs], DT, tag="h", bufs=4)
                h_tiles[c] = h_sb
                for fb in range(FB):
                    p = ps1.tile([128, cs], dt.float32, tag="ps1", bufs=3)
                    for kc in range(KD):
                        if isinstance(x_sb, tuple) and len(x_sb) == KD:
                            xop = x_sb[kc][:]
                        elif isinstance(x_sb, tuple):
                            xt_, k_ = (x_sb[0], kc) if kc < 2 else (x_sb[1], kc - 2)
                            xop = xt_[:, k_ * cs:(k_ + 1) * cs]
                        else:
                            xop = x_sb[:, kc * cs:(kc + 1) * cs]
                        nc.tensor.matmul(
                            p[:],
                            w1_slice(c, fb, kc),
                            xop,
                            start=(kc == 0),
                            stop=(kc == KD - 1),
                        )
                    nc.scalar.activation(
                        h_sb[:, fb, :],
                        p[:],
                        mybir.ActivationFunctionType.Relu,
                        bias=b1_sb[:, fb:fb + 1],
                        scale=1.0,
                    )

            def do_mm1_fp8(c):
                # d<256 half of the contraction as one DoubleRow fp8 matmul
                # per (fb, token-half); d>=256 half in bf16 with weights
                # pre-scaled by 2^15 to match the fp8 product scale
                # (32*1024); the activation divides the sum back out.
                cs = chunks[c]
                full = c in f8full_ids
                x_hi = x_tiles[c]       # [128, 2*cs] bf16: kc2,kc3
                x_lo = None if full else x8_tiles[c]  # [128, 2, cs] fp8
                m2 = c in m2_ids
                if m2:
                    # h split by dtype: fb0-3 as fp8*32 (feeds mm2's DR
                    # half), fb4-7 bf16
                    h8_sb = sbpool.tile([128, 4, cs], dt.float8e4, tag="h8",
                                        bufs=2)
                    hb_sb = sbpool.tile([128, 4, cs], DT, tag="hb", bufs=2)
                    h_tiles[c] = (h8_sb, hb_sb)
                else:
                    h_sb = sbpool.tile([128, FB, cs], DT, tag="h", bufs=4)
                    h_tiles[c] = h_sb
                ths = [(t0, min(256, cs - t0)) for t0 in range(0, cs, 256)]
                for fb in range(FB):
                    p = ps1.tile([128, cs], dt.float32, tag="ps1", bufs=3)
                    if full:
                        # whole contraction as two chained DoubleRow fp8
                        for hf in range(2):
                            nc.tensor.matmul(
                                p[:],
                                w18f_sb[:, fb, 2 * hf:2 * hf + 2],
                                x8f_sb[:, 2 * hf:2 * hf + 2, :],
                                start=(hf == 0), stop=(hf == 1),
                                perf_mode=mybir.MatmulPerfMode.DoubleRow,
                                skip_group_check=True,
                            )
                        nc.scalar.activation(
                            h_sb[:, fb, :], p[:],
                            mybir.ActivationFunctionType.Relu,
                            bias=b1a_sb[:, fb:fb + 1], scale=1.0 / 32768.0)
                        continue
                    # bf16 half FIRST, DR fp8 accumulating after: the
                    # reverse order (DR with start, bf16 accumulating)
                    # produces wrong psum contents on hardware
                    for kc in range(2):
                        nc.tensor.matmul(
                            p[:],
                            w1ahi_sb[:, fb, kc],
                            x_hi[:, kc * cs:(kc + 1) * cs],
                            start=(kc == 0), stop=False,
                            skip_group_check=True,
                        )
                    for ti, (t0, tw) in enumerate(ths):
                        nc.tensor.matmul(
                            p[:, t0:t0 + tw],
                            w18_sb[:, fb],
                            x_lo[:, :, t0:t0 + tw],
                            start=False, stop=(ti == len(ths) - 1),
                            perf_mode=mybir.MatmulPerfMode.DoubleRow,
                            skip_group_check=True,
                        )
                    if m2 and fb < 4:
                        # h8 = relu(pre*32): fold SH into the act scale
                        # (relu is positively homogeneous); bias = b1*32
                        nc.scalar.activation(
                            h8_sb[:, fb, :],
                            p[:],
                            mybir.ActivationFunctionType.Relu,
                            bias=b1a8_sb[:, fb:fb + 1],
                            scale=32.0 / 32768.0,
                        )
                    elif m2:
                        nc.scalar.activation(
                            hb_sb[:, fb - 4, :],
                            p[:],
                            mybir.ActivationFunctionType.Relu,
                            bias=b1a_sb[:, fb:fb + 1],
                            scale=1.0 / 32768.0,
                        )
                    else:
                        nc.scalar.activation(
                            h_sb[:, fb, :],
                            p[:],
                            mybir.ActivationFunctionType.Relu,
                            bias=b1a_sb[:, fb:fb + 1],
                            scale=1.0 / 32768.0,
                        )

            def do_mm2_fp8(c):
                # f<512 contraction half as DoubleRow fp8 (h8*32 x w2*1024),
                # f>=512 in bf16 with w2 pre-scaled 2^15; psum = 2^15 * y,
                # compensated by host-pre-scaled gates for these blocks.
                cs = chunks[c]
                h8_sb, hb_sb = h_tiles.pop(c)
                for tb in range(cs // 128):
                    blk = offs[c] // 128 + tb
                    r0 = offs[c] + tb * 128
                    p2 = ps2.tile([128, 512], dt.float32, tag="ps2", bufs=5)
                    for k in range(4):  # bf16 first (fb4-7)
                        nc.tensor.matmul(
                            p2[:],
                            hb_sb[:, k, tb * 128:(tb + 1) * 128],
                            w2hi_sb[:, k],
                            start=(k == 0), stop=False,
                            skip_group_check=True,
                        )
                    for j in range(2):
                        for q in range(2):
                            nc.tensor.matmul(
                                p2[:, q * 256:(q + 1) * 256],
                                h8_sb[:, 2 * j:2 * j + 2, tb * 128:(tb + 1) * 128],
                                w28_sb[:, j, :, q * 256:(q + 1) * 256],
                                start=False, stop=(j == 1 and q == 1),
                                perf_mode=mybir.MatmulPerfMode.DoubleRow,
                                skip_group_check=True,
                            )
                    o_sb = sbpool.tile([128, 512], DT, tag="o", bufs=16)
                    nc.vector.tensor_scalar_mul(
                        o_sb[:], p2[:], g_sb[:, blk:blk + 1]
                    )
                    nc.sync.dma_start(out=y_d[r0:r0 + 128, :], in_=o_sb[:])

            def do_mm2(c):
                cs = chunks[c]
                h_sb = h_tiles.pop(c)
                last_chunk = c == len(chunks) - 1
                for tb in range(cs // 128):
                    blk = offs[c] // 128 + tb
                    r0 = offs[c] + tb * 128
                    if not (last_chunk and tb == cs // 128 - 1):
                        p2 = ps2.tile([128, 512], dt.float32, tag="ps2", bufs=5)
                        for fb in range(FB):
                            nc.tensor.matmul(
                                p2[:],
                                h_sb[:, fb, tb * 128:(tb + 1) * 128],
                                w2_slice(c, fb),
                                start=(fb == 0),
                                stop=(fb == FB - 1),
                            )
                        o_sb = sbpool.tile([128, 512], DT, tag="o", bufs=16)
                        nc.vector.tensor_scalar_mul(
                            o_sb[:], p2[:], g_sb[:, blk:blk + 1]
                        )
                        nc.sync.dma_start(out=y_d[r0:r0 + 128, :], in_=o_sb[:])
                    else:
                        # final 128-token block: column-split mm2 into four
                        # quarters so the gate-scale + store of earlier
                        # quarters overlap mm2 of later ones, shrinking the
                        # post-last-matmul tail (store descriptor-gen is
                        # ~600ns serial per ring, so alternate rings)
                        for q, eng in ((0, nc.sync), (1, nc.scalar),
                                       (2, nc.sync), (3, nc.scalar)):
                            # reuse the regular ps2 slots ([128,512] tag) so
                            # PSUM stays within the 8-bank budget
                            p2 = ps2.tile([128, 512], dt.float32, tag="ps2", bufs=5)
                            for fb in range(FB):
                                nc.tensor.matmul(
                                    p2[:, 0:128],
                                    h_sb[:, fb, tb * 128:(tb + 1) * 128],
                                    w2_slice(c, fb, q * 128, (q + 1) * 128),
                                    start=(fb == 0),
                                    stop=(fb == FB - 1),
                                )
                            o_sb = sbpool.tile([128, 128], DT, tag="oh", bufs=4)
                            nc.vector.tensor_scalar_mul(
                                o_sb[:], p2[:, 0:128], g_sb[:, blk:blk + 1]
                            )
                            eng.dma_start(
                                out=y_d[r0:r0 + 128, q * 128:(q + 1) * 128],
                                in_=o_sb[:],
                            )

            # software pipeline: mm1 runs one chunk ahead of mm2, so the
            # first mm2's w2a dependency has ~2 chunk-times of DMA slack
            def do_mm2_any(c):
                if c in m2_ids:
                    do_mm2_fp8(c)
                else:
                    do_mm2(c)

            nchunks = len(chunks)
            for c in range(nchunks):
                if c in fp8_ids:
                    do_mm1_fp8(c)
                else:
                    do_mm1(c)
                if c >= 1:
                    do_mm2_any(c - 1)
            do_mm2_any(nchunks - 1)
    nc.compile()
    return nc


def _install_ntff_hook():
    """Register the axon NTFF profiling hook that run_bass_kernel_spmd
    (trace=True) looks for under antenv.axon_hooks; this container's antenv
    lacks that module, so recreate it via ctypes against libaxon_pjrt.so."""
    import sys, types, ctypes, contextlib

    if "antenv.axon_hooks" in sys.modules:
        return
    try:
        lib = ctypes.CDLL("/opt/axon/libaxon_pjrt.so")
    except OSError:
        return
    if not hasattr(lib, "axon_start_nrt_profile"):
        return
    lib.axon_start_nrt_profile.argtypes = [ctypes.POINTER(ctypes.c_int64), ctypes.c_size_t]
    lib.axon_start_nrt_profile.restype = ctypes.c_int64
    lib.axon_stop_nrt_profile.argtypes = [ctypes.c_char_p]
    lib.axon_stop_nrt_profile.restype = ctypes.c_int64

    @contextlib.contextmanager
    def _hook(output_dir, device_ids):
        import jax

        jax.devices()
        if device_ids:
            ids = (ctypes.c_int64 * len(device_ids))(*device_ids)
            rc = lib.axon_start_nrt_profile(ids, len(device_ids))
        else:
            rc = lib.axon_start_nrt_profile(None, 0)
        if rc != 0:
            raise RuntimeError(f"axon_start_nrt_profile rc={rc}")
        try:
            yield
        finally:
            n = lib.axon_stop_nrt_profile(str(output_dir).encode())
            print(f"profile: {n} ntff file(s) written to {output_dir}")

    mod = types.ModuleType("antenv.axon_hooks")
    _holder = {"h": _hook}
    mod.set_axon_ntff_profile_hook = lambda h: _holder.__setitem__("h", h)
    mod.get_axon_ntff_profile_hook = lambda: _holder["h"]
    sys.modules["antenv.axon_hooks"] = mod

    # avoid the S3/Fish artifact upload in the trace post-processing path
    import concourse.bass_utils as bu

    bu.upload_artifacts = lambda tmpdir: str(tmpdir)


def _pick_nt0(counts):
    """Smallest NT0 (multiple of 128) such that the overflow of every
    expert beyond NT0 fits in the 8 per-core 128-token top-up slots.
    Compare against the no-top-up template (pad all to max count)."""
    cmax = int(counts.max())
    nt_plain = max(512, -(-cmax // 128) * 128)
    best = None
    for nt0 in range(512, nt_plain + 128, 128):
        need = sum(-(-max(0, int(c) - nt0) // NT1) for c in counts)
        if need <= N_CORES:
            best = nt0
            break
    if best is None or best + NT1 >= nt_plain + NT1:
        best = nt_plain  # top-ups unused (gate=0 padding)
    return best


def kernel(**inputs):
    from concourse.bass_utils import run_bass_kernel_spmd
    import ml_dtypes

    if TRACE:
        _install_ntff_hook()

    x = np.asarray(inputs["x"], np.float32)
    w1 = np.asarray(inputs["w1"], np.float32)
    b1 = np.asarray(inputs["b1"], np.float32)
    w2 = np.asarray(inputs["w2"], np.float32)
    b2 = np.asarray(inputs["b2"], np.float32)
    wg = np.asarray(inputs["wg"], np.float32)
    bg = np.asarray(inputs["bg"], np.float32)

    T = x.shape[0] * x.shape[1]
    xf = x.reshape(T, D)

    # ---- host gating (fp64): logits -> top-2 (jax.lax.top_k tie order:
    # lower index wins -> stable argsort on -logits) -> softmax over top-2.
    logits = xf.astype(np.float64) @ wg.astype(np.float64) + bg.astype(np.float64)
    order = np.argsort(-logits, axis=1, kind="stable")
    top_idx = order[:, :TOP_K]                      # [T, K]
    top_vals = np.take_along_axis(logits, top_idx, axis=1)
    gwts = np.exp(top_vals - top_vals.max(axis=1, keepdims=True))
    gwts = gwts / gwts.sum(axis=1, keepdims=True)   # [T, K]

    # ---- dispatch: sort slots (t, k) by expert; per-expert contiguous runs.
    flat_expert = top_idx.ravel()                   # slot s = t*K + k
    perm = np.argsort(flat_expert, kind="stable")   # slots grouped by expert
    counts = np.bincount(flat_expert, minlength=E)
    cum = np.concatenate([[0], np.cumsum(counts)])
    slot_tok = perm // TOP_K                        # token of each sorted slot
    gates_sorted = gwts.ravel()[perm].astype(np.float32)

    NT0 = _pick_nt0(counts)
    NT = NT0 + NT1
    NTG = NT // 128

    # ---- mixed precision: per core, sort slot0 by gate descending; the
    # last NF slots (lowest gates, all g<=0.5) run mm1's d<256 half in fp8
    # DoubleRow -- quantization noise there is damped by the gate weight.
    # Measured rel-err 1.707e-2 at NF=1920 vs the 2e-2 gate (bf16 baseline
    # 3.9e-3; NF=2048+ would cross 2.2e-2).
    core_ord = []
    sec_min = NT
    for c in range(N_CORES):
        n0c = min(int(counts[c]), NT0)
        g = gates_sorted[cum[c]:cum[c] + n0c]
        og = np.argsort(-g, kind="stable")
        core_ord.append(og)
        sec_min = min(sec_min, int((g <= 0.5).sum()))
    NF = 128 * (min(1920, sec_min) // 128)
    chunks, fp8_ids, m2_ids, f8full_ids = _chunk_plan(NT0, NF)
    offs = [sum(chunks[:i]) for i in range(len(chunks) + 1)]

    io_dtype = ml_dtypes.bfloat16
    w1_io = w1.astype(io_dtype)
    w2_io = w2.astype(io_dtype)

    # top-up assignment: expert e's slots beyond NT0, chopped into
    # 128-blocks, each block -> one core's slot1. record: (core, e, lo, n)
    topups = []
    next_core = 0
    for e in range(E):
        n = int(counts[e])
        for lo in range(NT0, n, NT1):
            nb = min(NT1, n - lo)
            assert next_core < N_CORES, "top-up slots exhausted"
            topups.append((next_core, e, lo, nb))
            next_core += 1
    topup_by_core = {c: (e, lo, nb) for (c, e, lo, nb) in topups}

    def permute_x(xt):
        # xt [D, NT] -> [128, KD*NT]: per chunk, (kc, token) contiguous
        xr = xt.reshape(KD, 128, NT)
        parts = [
            xr[:, :, offs[c]:offs[c + 1]].transpose(1, 0, 2).reshape(128, -1)
            for c in range(len(chunks))
        ]
        return np.ascontiguousarray(np.concatenate(parts, axis=1))

    def pack_w1(e):
        # [128, FB*KD*128] fb-major: col = fb*KD*128 + kc*128 + j
        w = w1_io[e].reshape(KD, 128, FB, 128)       # [kc, p, fb, j]
        return np.ascontiguousarray(
            w.transpose(1, 2, 0, 3).reshape(128, FB * KD * 128))

    def pack_w2(e):
        return np.ascontiguousarray(
            w2_io[e].reshape(FB, 128, D).transpose(1, 0, 2).reshape(128, FB * D))

    def pack_b1(e):
        return np.ascontiguousarray(b1[e].reshape(FB, 128).T)

    m_fp8 = len(fp8_ids)
    SX, SW = 32.0, 1024.0  # exact powers of two; bf16 half carries 2^15
    SH = 32.0              # h scale for the fp8-mm2 chunk

    def pack_w28(e):
        # [128, (j,i,d)] e4m3: w2[(2j+i)*128+p, d]*SW for f<512
        w = w2[e][:512].reshape(2, 2, 128, D)        # [j, i, p, d]
        return np.ascontiguousarray(
            (w.transpose(2, 0, 1, 3) * SW).reshape(128, 2048)
        ).astype(ml_dtypes.float8_e4m3)

    def pack_w2hi(e):
        # [128, (k,d)] bf16: w2[512+k*128+p, d]*2^15
        w = w2[e][512:].reshape(4, 128, D)           # [k, p, d]
        return np.ascontiguousarray(
            (w.transpose(1, 0, 2) * (SH * SW)).reshape(128, 4 * D)
        ).astype(io_dtype)

    def pack_w18(e):
        # [128, FB*2*128] e4m3: col (fb, i, j) = w1[i*128+p, fb*128+j]*SW
        w = w1[e][:256].reshape(2, 128, FB, 128)     # [i, p, fb, j]
        return np.ascontiguousarray(
            (w.transpose(1, 2, 0, 3) * SW).reshape(128, FB * 256)
        ).astype(ml_dtypes.float8_e4m3)

    def pack_w1ahi(e):
        # [128, FB*2*128] bf16: col (fb, kc, j) = w1[(kc+2)*128+p, ...]*2^15
        w = w1[e][256:].reshape(2, 128, FB, 128)
        return np.ascontiguousarray(
            (w.transpose(1, 2, 0, 3) * (SX * SW)).reshape(128, FB * 256)
        ).astype(io_dtype)

    in_maps = []
    for c in range(N_CORES):
        n0 = min(int(counts[c]), NT0)
        toks0 = slot_tok[cum[c]:cum[c] + n0][core_ord[c]]
        xt = np.zeros((D, NT), io_dtype)
        xt[:, :n0] = xf[toks0].astype(io_dtype).T
        gate = np.zeros(NT, np.float32)
        gate[:n0] = gates_sorted[cum[c]:cum[c] + n0][core_ord[c]]
        # fp8-mm2 chunk: its psum carries an extra 2^15 factor; fold the
        # compensation into the gate values for those blocks
        for cid in m2_ids:
            gate[offs[cid]:offs[cid + 1]] /= SH * SW
        if c in topup_by_core:
            te, lo, nb = topup_by_core[c]
            tt = slot_tok[cum[te] + lo:cum[te] + lo + nb]
            xt[:, NT0:NT0 + nb] = xf[tt].astype(io_dtype).T
            gate[NT0:NT0 + nb] = gates_sorted[cum[te] + lo:cum[te] + lo + nb]
            eb = te
        else:
            eb = 0  # unused slot1: gate=0 rows, any weights
        im = {
            "xp": permute_x(xt),
            "w1a": pack_w1(c), "w2a": pack_w2(c), "b1a": pack_b1(c),
            "w1b": pack_w1(eb), "w2b": pack_w2(eb), "b1b": pack_b1(eb),
            "gate2": np.ascontiguousarray(gate.reshape(NTG, 128).T),
        }
        if m_fp8:
            # x8 per fp8 chunk: [128, (i, t)] = x[d=i*128+p, tok]*SX, fp32
            # source (not the bf16 xt) to avoid double rounding
            x8parts = []
            for cid in fp8_ids:
                cs8 = chunks[cid]
                tk = toks0[offs[cid]:offs[cid] + cs8]
                xc = np.zeros((256, cs8), np.float32)
                xc[:, :len(tk)] = xf[tk].T[:256] * SX
                xr = xc.reshape(2, 128, cs8)                     # [i, p, t]
                x8parts.append(xr.transpose(1, 0, 2).reshape(128, 2 * cs8))
            im["xp8"] = np.ascontiguousarray(
                np.concatenate(x8parts, axis=1)).astype(ml_dtypes.float8_e4m3)
            im["w18"] = pack_w18(c)
            im["w1ahi"] = pack_w1ahi(c)
        if m2_ids:
            im["w28"] = pack_w28(c)
            im["w2hi"] = pack_w2hi(c)
            im["b1a8"] = np.ascontiguousarray(
                (b1[c] * SH).reshape(FB, 128).T.astype(np.float32))
        if f8full_ids:
            cid = f8full_ids[0]
            tk = toks0[offs[cid]:offs[cid] + 256]
            xc = np.zeros((512, 256), np.float32)
            xc[:, :len(tk)] = xf[tk].T * SX
            im["xp8f"] = np.ascontiguousarray(
                xc.reshape(4, 128, 256).transpose(1, 0, 2).reshape(128, 1024)
            ).astype(ml_dtypes.float8_e4m3)
            wf = w1[c].reshape(4, 128, FB, 128)      # [i, p, fb, j]
            im["w18f"] = np.ascontiguousarray(
                (wf.transpose(1, 2, 0, 3) * SW).reshape(128, FB * 512)
            ).astype(ml_dtypes.float8_e4m3)
        in_maps.append(im)

    def run_device():
        key = (NT0, NF)
        if key not in _PROGRAM_CACHE:
            _PROGRAM_CACHE[key] = _build_program(NT0, NF)
        nc = _PROGRAM_CACHE[key]
        res = run_bass_kernel_spmd(nc, in_maps, list(range(N_CORES)), trace=TRACE)
        if TRACE and res.exec_time_ns is not None:
            print(f"HW exec time: {res.exec_time_ns} ns")
        return [res.results[c]["y"] for c in range(N_CORES)]

    try:
        try:
            y_cores = run_device()
        except Exception:
            # transient device errors (e.g. NRT exec-unit unrecoverable)
            # are usually gone on retry with a freshly built program
            _PROGRAM_CACHE.clear()
            y_cores = run_device()
    except Exception as exc:
        # last resort: identical math on the host so the result is still
        # correct even if the accelerator path is down
        import sys
        print(f"device path failed twice ({exc!r}); computing FFN on host",
              file=sys.stderr)
        out_slots = np.zeros((T * TOP_K, D), np.float32)
        for e in range(E):
            n = int(counts[e])
            toks = slot_tok[cum[e]:cum[e] + n]
            h = np.maximum(xf[toks] @ w1[e] + b1[e], 0.0)
            y = (h @ w2[e]) * gates_sorted[cum[e]:cum[e] + n, None]
            out_slots[perm[cum[e]:cum[e] + n]] = y.astype(np.float32)
        out = out_slots.reshape(T, TOP_K, D).sum(axis=1)
        combine = np.zeros((T, E), np.float32)
        np.put_along_axis(combine, top_idx, gwts.astype(np.float32), axis=1)
        out += combine @ b2
        return out.reshape(B, S, D).astype(np.float32)

    # ---- unshard: scatter slots back, sum the K slots per token, add b2.
    out_slots = np.zeros((T * TOP_K, D), np.float32)
    for c in range(N_CORES):
        n0 = min(int(counts[c]), NT0)
        sl = np.arange(cum[c], cum[c] + n0)[core_ord[c]]
        out_slots[perm[sl]] = y_cores[c][:n0].astype(np.float32)
    for (c, e, lo, nb) in topups:
        out_slots[perm[cum[e] + lo:cum[e] + lo + nb]] = \
            y_cores[c][NT0:NT0 + nb].astype(np.float32)
    out = out_slots.reshape(T, TOP_K, D).sum(axis=1)

    # combine @ b2 (gate-weighted expert output biases)
    combine = np.zeros((T, E), np.float32)
    np.put_along_axis(combine, top_idx, gwts.astype(np.float32), axis=1)
    out += combine @ b2

    return out.reshape(B, S, D).astype(np.float32)

